# revision 94
# baseline (speedup 1.0000x reference)
"""CanonCausalMultiheadAttn Trainium2 kernel (fp8 DoubleRow version).

Sharding: 8 cores = 2 (batch) x 4 (kv-head groups). Core c handles batch
c//4 and kv-group g=c%4 (q heads 4g..4g+3, kv head g). w_q/w_k/w_v are
column-sharded by head group, w_o row-sharded; each core emits a partial
[S, D] output (bf16) which the host sums over the 4 groups of its batch.

The four heads of each q-tile are software-pipelined into one flat
(head, k-tile) sequence with a deep (LAG=14) pending-probs queue, so the
in-order PE queue never stalls on the exp (ACT) -> causal-mask (Pool)
producer chain; the previous q-tile's output projection is interleaved as
PE filler. All DMAs serialize through one shared HWDGE (~625ns dispatch
each), so hi/lo fp8 planes are PACKED into single DRAM tensors and output
tiles ship as one wide DMA per 128-token chunk. A short chain of warm-up
matmuls primes the PE p-state ramp (half clock until 3us continuously
busy) while the first input DMAs are in flight.

Per-core dataflow (transposed [feature, token] layout; v transposed on PE):
  qkvT[f, t] = w_qkv[:, f].T @ hT[:, t]   -- fp8e4m3 DoubleRow matmuls with
      3-term hi/lo compensation (w_hi.h_hi + w_hi.h_lo + w_lo.h_hi), pr-major
      so each hidden chunk is fully consumed as it lands.
  conv: depthwise causal taps in bf16 on DVE (tensor_scalar products,
      tensor_tensor sums); conv weights stay f32.
  scores.T[k, q] = kT.T @ qT (bf16) -> exp on ACT (scale folds the fp8
      pre-scales; bias -2ln2 keeps fp16 column sums in range)
  causal: k-tiles with k0 <= q_end only; diagonal tiles masked in-place
      by an affine_select on the (otherwise idle) Pool engine.
  attT[dh, q] += v_nat[k,:].T @ probsT  (bf16)
  colsum via DVE adds in fp16 (2x DVE mode), partition-reduced by one
      fp16 ones-matmul (ones=4.0 folds the attT scale correction).
  attT stored as fp8 hi+lo; out[t, d] = attT.T @ w_o_rows via 3-term DR,
      shipped bf16 (divided by 1024x net scale on the host).

Scales: w_qkv and w_o are pre-scaled x64 into fp8 (e4m3 = IEEE variant,
  max finite 240); hidden stays x1. qkv = 64x, scores = 4096 s (folded
  into exp scale), probs = p/4 (exp bias), att = 16*Sum p v, colsum = p/4
  summed, ones=4.0 => attT = 16*attended, out = 1024*true. Host divides.
"""

import numpy as np
import ml_dtypes
from collections import deque
from contextlib import ExitStack

import concourse.bass as bass
import concourse.tile as tile
import concourse.mybir as mybir
from concourse.bass import ds, ts
from concourse.bass_utils import run_bass_kernel_spmd
from concourse.masks import make_identity

BF16 = mybir.dt.bfloat16
F16 = mybir.dt.float16
F32 = mybir.dt.float32
FP8 = mybir.dt.float8e4
DR = mybir.MatmulPerfMode.DoubleRow
P = 128
S = 2048          # sequence length
D = 2048          # d_model
NF = 6            # feature chunks of 128: 4 q heads, 1 k, 1 v
NPR = 8           # DR contraction pairs over d_model (2048 = 8*256)
NQT = S // 512    # 4 query tiles of 512
NTT = S // 512    # 4 token tiles of 512
WSCALE = 64.0     # fp8 pre-scale on w_qkv and w_o (e4m3 max is 240)
ISQ = 1.0 / np.sqrt(128.0)
EXP_SCALE = ISQ / (WSCALE * WSCALE)   # scores PSUM holds 4096*s
EXP_BIAS = float(-2.0 * np.log(2.0))  # probs = p/4 (fp16 colsum headroom)
OUT_DIV = 1024.0  # 16 (attT) * 64 (w_o)
WARM = 7          # PE p-state warm-up matmuls
MULT = mybir.AluOpType.mult
ADD = mybir.AluOpType.add

_CACHE = {}


def _legalize_waits(nc):
    """Split multi-wait sync_info into preceding single-wait engine NOPs.

    The walrus codegen in this container accepts at most ONE sync wait per
    TPB instruction ("Too many sync wait commands"), but the Tile scheduler
    freely emits several. An engine executes its queue in order, so hoisting
    the extra waits onto NoOps right before the instruction is equivalent.
    """
    n = 0
    for f in nc.m.functions:
        for blk in f.blocks:
            out = []
            changed = False
            for inst in blk.instructions:
                si = inst.sync_info
                if (si is not None and si.on_wait and len(si.on_wait) > 1
                        and str(inst.engine) != "EngineType.Unassigned"):
                    waits = list(si.on_wait)
                    for w in waits[:-1]:
                        out.append(mybir.InstNoOp(
                            name=f"I-wf{n}", engine=inst.engine, ins=[],
                            outs=[],
                            sync_info=mybir.SyncInfo(on_wait=[w],
                                                     on_update=[])))
                        n += 1
                    si.on_wait = [waits[-1]]
                    changed = True
                out.append(inst)
            if changed:
                blk.instructions = out
    return n


def _build(legalize=True):
    key = "nc" if legalize else "nc_raw"
    if key in _CACHE:
        return _CACHE[key]
    nc = bass.Bass("TRN2", target_bir_lowering=False, debug=False)

    # hi/lo fp8 planes packed into single DRAM tensors: every DMA dispatch
    # serializes through one shared HWDGE (~625ns), so fewer+wider wins
    h_d = nc.dram_tensor("h", [P, NTT, NPR, 2, 2, 512], FP8,
                         kind="ExternalInput").ap()
    w_d = nc.dram_tensor("w", [P, NF, NPR, 2, 2, P], FP8,
                         kind="ExternalInput").ap()
    wo_d = nc.dram_tensor("wo", [P, 2, 2, 2, D], FP8,
                          kind="ExternalInput").ap()
    cw_d = nc.dram_tensor("conv_w", [P, NF * 4], F32,
                          kind="ExternalInput").ap()
    out_d = nc.dram_tensor("out", [S, D], BF16, kind="ExternalOutput").ap()

    out_v = out_d.rearrange("(po pi) d -> pi po d", pi=P)      # [128,16,2048]

    with tile.TileContext(nc) as tc, ExitStack() as ctx:
        const = ctx.enter_context(tc.tile_pool(name="const", bufs=1))
        p_ht = ctx.enter_context(tc.tile_pool(name="ht", bufs=2))
        p_work = ctx.enter_context(tc.tile_pool(name="work", bufs=3))
        p_probs = ctx.enter_context(tc.tile_pool(name="probs", bufs=6))
        p_out = ctx.enter_context(tc.tile_pool(name="outp", bufs=4))
        ps2 = ctx.enter_context(tc.tile_pool(name="ps2", bufs=2, space="PSUM"))
        ps_s = ctx.enter_context(tc.tile_pool(name="ps_s", bufs=3,
                                              space="PSUM"))
        ps3 = ctx.enter_context(tc.tile_pool(name="ps3", bufs=2, space="PSUM"))
        ps1 = ctx.enter_context(tc.tile_pool(name="ps1", bufs=1, space="PSUM"))

        # --- constants / persistent tensors ---
        # tiny warm operand memset FIRST on Pool (~100ns) so the PE p-state
        # warm-up starts ~1us before make_identity would allow
        wtiny = const.tile([P, P], BF16, tag="wtiny")
        nc.gpsimd.memset(wtiny, 0.5)
        ident = const.tile([P, P], BF16, tag="ident")
        make_identity(nc, ident)
        wscr = const.tile([P, 512], BF16, tag="wscr")
        nc.vector.memset(wscr, 0.5)
        # PE p-state warm-up: the tensor engine runs at half clock until it
        # has been continuously busy 3us; burn that ramp on dummies while
        # the first input DMAs are still in flight.
        wps = ps_s.tile([P, 512], F32, tag="s", name="warm")
        for _ in range(8):
            nc.tensor.matmul(wps[:, 0:P], lhsT=wtiny, rhs=wtiny,
                             start=True, stop=True)
        for _ in range(WARM):
            nc.tensor.matmul(wps, lhsT=wtiny, rhs=wscr, start=True, stop=True)
        cw0 = const.tile([P, NF * 4], F32, tag="cw0")
        cw = const.tile([P, NF * 4], F32, tag="cw")
        wq = const.tile([P, NF, NPR, 2, 2, P], FP8, tag="wq")
        wo = const.tile([P, 2, 2, 2, D], FP8, tag="wo")
        # raw (pre-conv) qkv.T in bf16 (64x scale), 3 leading zero columns so
        # the causal conv taps can read t-3..t-1 without edge cases
        qkvf = const.tile([P, NF, S + 3], BF16, tag="qkvf")
        nc.gpsimd.memset(qkvf[:, :, 0:3], 0.0)
        qkvb = const.tile([P, NF, S], BF16, tag="qkvb")    # conv'd qkv.T
        vnat = const.tile([P, 16, P], BF16, tag="vnat")    # v in [token, dh]
        atth = const.tile([P, 4, S], FP8, tag="atth")      # attT hi per head
        attl = const.tile([P, 4, S], FP8, tag="attl")      # attT lo per head
        ones2 = const.tile([P, P], F16, tag="ones2")
        nc.vector.memset(ones2, 4.0)
        ebias = const.tile([P, 1], F32, tag="ebias")
        nc.vector.memset(ebias, EXP_BIAS)

        obuf = {}  # t4 -> wide bf16 output tile (one DMA per token chunk)

        def oproj_trio(tt16, op, dt, pr_, k0):
            # atth terms first: the attl piece lands one DVE op later in
            # the finalize chain, so leading with atth starts ~190ns sooner
            for k, (lhs, hl) in enumerate(
                    ((atth, 0), (atth, 1), (attl, 0))):
                nc.tensor.matmul(
                    op,
                    lhsT=lhs[:, ds(2 * pr_, 2), ds(tt16 * P, P)],
                    rhs=wo[:, hl, pr_, :, ds(dt * 512, 512)],
                    start=(k0 + k == 0), stop=(k0 + k == 5),
                    perf_mode=DR)

        def o_proj_final_pr0(t4):
            # head-0/1 trios of a final chunk: eligible as soon as the
            # first head pair is normalized, used as PE filler while the
            # last head's denominator drains on DVE
            tt16 = (NQT - 1) * 4 + t4
            ops = {}
            for dt in range(4):
                if dt % 2 == 1:
                    ops[dt] = ps_s.tile([P, 512], F32, tag="s", name="op")
                else:
                    ops[dt] = ps2.tile([P, 512], F32, tag="proj", name="op")
                oproj_trio(tt16, ops[dt], dt, 0, 0)
            return ops

        def o_proj_chunk(qt, t4, final=False, dts=(0, 1, 2, 3),
                         pre_ops=None):
            # output projection for one token-128-tile of q-tile qt; the
            # PSUM->SBUF copies land in one wide bf16 tile which ships as a
            # SINGLE DMA per token chunk (HWDGE dispatch is the scarce
            # resource, not DMA bandwidth)
            tt16 = qt * 4 + t4
            if t4 not in obuf:
                obuf[t4] = p_out.tile([P, 4, 512], BF16, tag="ob", name="ob")
            ob = obuf[t4]

            def trio(op, dt, pr_, k0):
                oproj_trio(tt16, op, dt, pr_, k0)

            for dt in dts:
                if pre_ops is not None:
                    op = pre_ops[dt]
                else:
                    if final and dt % 2 == 1:
                        op = ps_s.tile([P, 512], F32, tag="s", name="op")
                    else:
                        op = ps2.tile([P, 512], F32, tag="proj", name="op")
                    trio(op, dt, 0, 0)
                trio(op, dt, 1, 3)
                if final:
                    # alternate engines so the last chunk's copies overlap;
                    # ship in pieces so earlier DMAs overlap later matmuls
                    if dt == 3 and t4 == 3:
                        # the very last copy splits across ACT+DVE halves
                        # running in parallel (~390ns instead of 612)
                        nc.scalar.copy(ob[:, dt, ds(0, 256)],
                                       op[:, ds(0, 256)])
                        nc.vector.tensor_copy(ob[:, dt, ds(256, 256)],
                                              op[:, ds(256, 256)])
                    elif dt % 2 == 0:
                        nc.vector.tensor_copy(ob[:, dt, :], op)
                    else:
                        nc.scalar.copy(ob[:, dt, :], op)
                    if dt == 1:
                        nc.sync.dma_start(out_v[:, tt16, ds(0, 1024)],
                                          ob[:, ds(0, 2)])
                    elif dt == 2:
                        nc.sync.dma_start(out_v[:, tt16, ds(1024, 512)],
                                          ob[:, ds(2, 1)])
                    elif dt == 3:
                        nc.sync.dma_start(out_v[:, tt16, ds(1536, 512)],
                                          ob[:, ds(3, 1)])
                elif dt == 3:
                    nc.scalar.copy(ob[:, dt, :], op)
                else:
                    nc.vector.tensor_copy(ob[:, dt, :], op)
            if dts[-1] == 3:
                if not final:
                    nc.sync.dma_start(out_v[:, tt16, :], ob)
                del obuf[t4]

        def attn_B(qt, fillers=None, drain_filler=None):
            # attention for q-tile qt (needs token tiles <= qt). The four
            # heads are software-pipelined into ONE flat (h, kt) sequence:
            # scores of head h+1 are emitted while head h's attended matmuls
            # drain, so the in-order PE queue never stalls on the
            # exp (ACT) -> mask (Pool) producer chain. The previous q-tile's
            # output projection is interleaved as additional PE filler.
            nk = 4 * (qt + 1)
            LAG = 14
            state = {}  # h -> (att, colsum)
            fin = {}    # h -> (att, colsum) awaiting denominator finalize
            fin_q = deque()  # [h, consumes-since-ready]
            pend = deque()
            pr_quad = None

            def consume():
                ch, ppr, px0, pkt = pend.popleft()
                att, colsum = state[ch] if ch in state else fin[ch]
                nc.tensor.matmul(
                    att[:, px0:512], lhsT=vnat[:, pkt, :],
                    rhs=ppr[:, px0:512],
                    start=(pkt == 0), stop=(pkt == nk - 1))
                # softmax denominator: accumulate exp'd probs on DVE
                # (partition dim reduced by ONE ones-matmul at the end)
                if pkt == 0:
                    nc.vector.tensor_copy(colsum, ppr)
                else:
                    nc.vector.tensor_add(
                        colsum[:, px0:512], colsum[:, px0:512],
                        ppr[:, px0:512])
                if pkt == nk - 1:
                    fin[ch] = state.pop(ch)
                    fin_q.append([ch, 0])

            def finalize(ch):
                att, colsum = fin[ch]
                smp = ps1.tile([P, 512], F32, tag="small")
                rec = p_work.tile([P, 512], F32, tag="rec")
                t16 = p_work.tile([P, 512], F16, tag="t16")
                if qt == NQT - 1 and ch >= 2:
                    # last heads before the final output projection: the
                    # whole normalization chain runs in 128-col pieces
                    # (ones-matmul and reciprocal included) so the first
                    # final o_proj trios start ~0.6us sooner
                    for pc in range(4):
                        c = ds(pc * P, P)
                        nc.tensor.matmul(smp[:, c], lhsT=ones2,
                                         rhs=colsum[:, c],
                                         start=True, stop=True)
                        nc.vector.reciprocal(rec[:, c], smp[:, c])
                        nc.vector.tensor_mul(t16[:, c], att[:, c], rec[:, c])
                        nc.vector.tensor_copy(
                            atth[:, ch, ds(qt * 512 + pc * P, P)], t16[:, c])
                        nc.vector.tensor_sub(
                            attl[:, ch, ds(qt * 512 + pc * P, P)], t16[:, c],
                            atth[:, ch, ds(qt * 512 + pc * P, P)])
                else:
                    nc.tensor.matmul(smp, lhsT=ones2, rhs=colsum,
                                     start=True, stop=True)
                    nc.vector.reciprocal(rec, smp)
                    nc.vector.tensor_mul(t16, att, rec)
                    nc.gpsimd.tensor_copy(atth[:, ch, ts(qt, 512)], t16)
                    nc.gpsimd.tensor_sub(attl[:, ch, ts(qt, 512)], t16,
                                         atth[:, ch, ts(qt, 512)])
                del fin[ch]

            for h in range(4):
                if fillers and h in fillers:
                    fillers[h]()
                state[h] = (ps3.tile([P, 512], F32, tag="att", name="att"),
                            p_work.tile([P, 512], F16, tag="colsum",
                                        name="colsum"))
                for kt in range(nk):
                    # shorter pending queue only near the END of the very
                    # last head: early kts keep the deep pipeline (PE ahead
                    # of exp), the tail still drains early so the final
                    # output projection starts with less latency
                    lag = 4 if (qt == NQT - 1 and h == 3
                                and kt >= nk - 6) else LAG
                    # previous q-tile's output projection emitted mid-head;
                    # on long tiles spread it at four points so the filler
                    # matches the exp-bound score/attend cadence
                    if qt > 0 and nk >= 12:
                        pts = (3, 6, 9, 11) if nk == 12 else (4, 8, 11, 14)
                        if kt in pts:
                            o_proj_chunk(qt - 1, h, dts=(pts.index(kt),))
                    elif qt > 0:
                        if kt == 4:
                            o_proj_chunk(qt - 1, h, dts=(0, 1))
                        elif kt == 7:
                            o_proj_chunk(qt - 1, h, dts=(2, 3))
                    j = kt - 4 * qt
                    x0 = j * P if j >= 0 else 0
                    F = 512 - x0
                    sp = ps_s.tile([P, 512], F32, tag="s")
                    nc.tensor.matmul(
                        sp[:, x0:512],
                        lhsT=qkvb[:, 4, ds(kt * P, P)],
                        rhs=qkvb[:, h, ds(qt * 512 + x0, F)],
                        start=True, stop=True,
                    )
                    if kt % 4 == 0:
                        pr_quad = p_probs.tile([P, 4, 512], BF16, tag="probs")
                    pr = pr_quad[:, kt % 4, :]
                    nc.scalar.activation(
                        pr[:, x0:512], sp[:, x0:512],
                        mybir.ActivationFunctionType.Exp,
                        scale=EXP_SCALE, bias=ebias)
                    if j >= 0:
                        # zero the k>q half of the diagonal tile in place
                        # (local col c vs partition p: keep iff c >= p)
                        nc.gpsimd.affine_select(
                            out=pr[:, x0:512], in_=pr[:, x0:512],
                            pattern=[[1, F]], base=0,
                            channel_multiplier=-1,
                            compare_op=mybir.AluOpType.is_ge, fill=0.0)
                    pend.append((h, pr, x0, kt))
                    thr = 1 if qt == NQT - 1 else 3
                    while len(pend) > lag:
                        consume()
                        for e in fin_q:
                            e[1] += 1
                        if fin_q and fin_q[0][1] >= thr:
                            finalize(fin_q.popleft()[0])
            while pend:
                consume()
            if drain_filler is not None:
                # PE work emitted ahead of the remaining finalizes, whose
                # ones-matmuls stall in-order on the DVE colsum chain
                drain_filler()
            while fin_q:
                finalize(fin_q.popleft()[0])

        # ------- Fused phases: per token tile: projection+conv, then the
        # attention q-tile that just became computable, then the (pipelined)
        # output projection of the previous q-tile.
        def ht_alloc():
            return p_ht.tile([P, NPR, 2, 2, 512], FP8, tag="ht", name="ht")

        def ht_dispatch(ht, tt, chunks):
            for c0, w_ in chunks:
                nc.sync.dma_start(ht[:, ds(c0, w_)],
                                  h_d[:, tt, ds(c0, w_)])

        def conv_fc(tt, fc, pp, dve_copy=False):
            t0 = tt * 512
            # pre-conv x (64x) -> bf16 for the DVE conv taps. When run
            # as attention filler, the copy goes on DVE because ACT is
            # clogged with exp tiles there.
            if dve_copy:
                nc.vector.tensor_copy(qkvf[:, fc, ds(3 + t0, 512)], pp)
            else:
                nc.scalar.copy(qkvf[:, fc, ds(3 + t0, 512)], pp)
            # conv taps: out[t] = x[t] + sum_k x[t+k-3]*w[k].
            # Products via tensor_scalar (4x DVE mode — the tensor-tensor
            # variant gets no fast mode), sums via bf16 tensor_tensor (2x)
            ca = p_work.tile([P, 512], BF16, tag="ctmpa", name="ca")
            cb = p_work.tile([P, 512], BF16, tag="ctmpb", name="cb")
            nc.vector.tensor_scalar(
                ca, qkvf[:, fc, ds(t0 + 0, 512)],
                cw[:, fc * 4 + 0: fc * 4 + 1], None, op0=MULT)
            nc.vector.tensor_scalar(
                cb, qkvf[:, fc, ds(t0 + 1, 512)],
                cw[:, fc * 4 + 1: fc * 4 + 2], None, op0=MULT)
            nc.vector.tensor_add(ca, ca, cb)
            nc.vector.tensor_scalar(
                cb, qkvf[:, fc, ds(t0 + 2, 512)],
                cw[:, fc * 4 + 2: fc * 4 + 3], None, op0=MULT)
            nc.vector.tensor_add(ca, ca, cb)
            # last tap's weight is pre-biased +1 on the host, folding the
            # residual x[t] into the same tensor_scalar product
            nc.vector.tensor_scalar(
                cb, qkvf[:, fc, ds(t0 + 3, 512)],
                cw[:, fc * 4 + 3: fc * 4 + 4], None, op0=MULT)
            nc.vector.tensor_add(qkvb[:, fc, ts(tt, 512)], ca, cb)

        def proj_fc(tt, fc, ht, dve_copy=False):
            pp = ps2.tile([P, 512], F32, tag="proj", name="pp")
            # pr-major: each hidden pr chunk is fully consumed (all three
            # hi/lo terms) as soon as it lands, minimizing startup stalls
            k = 0
            for pr_ in range(NPR):
                for whl, hhl in ((0, 0), (0, 1), (1, 0)):
                    nc.tensor.matmul(
                        pp, lhsT=wq[:, fc, pr_, whl], rhs=ht[:, pr_, hhl],
                        start=(k == 0), stop=(k == 3 * NPR - 1),
                        perf_mode=DR)
                    k += 1
            conv_fc(tt, fc, pp, dve_copy)

        def transp_v(tt):
            # v (fc=5) of this token tile -> natural [token, dh] layout
            trp = ps1.tile([P, 512], BF16, tag="small")
            for j in range(4):
                nc.tensor.transpose(trp[:, ds(j * P, P)],
                                    qkvb[:, 5, ds((tt * 4 + j) * P, P)],
                                    ident)
            nc.vector.tensor_copy(vnat[:, ds(tt * 4, 4), :], trp)

        prefetched = None
        for tt in range(NTT):
            if tt == 0:
                # The DMA engine pool executes one transfer at a time, so
                # the dispatch sequence below is a global priority schedule:
                # interleave hidden pr-chunks with the weight packs in
                # exactly the order the paired pr-major projection consumes
                # them (fc4+fc5 first, pr chunk by pr chunk).
                ht = ht_alloc()
                ht_dispatch(ht, 0, ((0, 1),))
                nc.sync.dma_start(wq[:, 4, ds(0, 4)], w_d[:, 4, ds(0, 4)])
                nc.sync.dma_start(wq[:, 5, ds(0, 4)], w_d[:, 5, ds(0, 4)])
                ht_dispatch(ht, 0, ((1, 1), (2, 2)))
                nc.sync.dma_start(wq[:, 4, ds(4, 4)], w_d[:, 4, ds(4, 4)])
                nc.sync.dma_start(wq[:, 5, ds(4, 4)], w_d[:, 5, ds(4, 4)])
                ht_dispatch(ht, 0, ((4, 2), (6, 2)))
                nc.sync.dma_start(cw0, cw_d)
                nc.vector.tensor_copy(cw, cw0)
                for fc in (0, 1, 2, 3):
                    nc.sync.dma_start(wq[:, fc], w_d[:, fc])
                prefetched = ht_alloc()
                ht_dispatch(prefetched, 1, ((0, 4), (4, 4)))
            else:
                ht = prefetched
                if tt == 1:
                    nc.sync.dma_start(wo, wo_d)
                if tt < NTT - 1:
                    prefetched = ht_alloc()
                    ht_dispatch(prefetched, tt + 1, ((0, 4), (4, 4)))

            if tt == 0:
                # fc4+fc5 chains interleaved pr-major: two PSUM chains
                # consume each hidden pr chunk at the DMA supply rate; the
                # remaining projections ride inside the attention blocks
                pps = {4: ps2.tile([P, 512], F32, tag="proj", name="pp"),
                       5: ps2.tile([P, 512], F32, tag="proj", name="pp")}
                ks = {4: 0, 5: 0}
                for pr_ in range(NPR):
                    for fc in (4, 5):
                        for whl, hhl in ((0, 0), (0, 1), (1, 0)):
                            nc.tensor.matmul(
                                pps[fc], lhsT=wq[:, fc, pr_, whl],
                                rhs=ht[:, pr_, hhl],
                                start=(ks[fc] == 0),
                                stop=(ks[fc] == 3 * NPR - 1), perf_mode=DR)
                            ks[fc] += 1
                    # pad known DMA-supply stalls with warm matmuls so the
                    # PE p-state ramp survives the bandwidth-gated stretch
                    for _ in range({1: 2, 3: 4}.get(pr_, 0)):
                        wpad = ps_s.tile([P, 512], F32, tag="s", name="wpad")
                        nc.tensor.matmul(wpad, lhsT=wtiny, rhs=wscr,
                                         start=True, stop=True)
                conv_fc(0, 4, pps[4])
                conv_fc(0, 5, pps[5])
                proj_fc(0, 0, ht)
                proj_fc(0, 1, ht)
            else:
                for fc in (0, 1, 2, 3):
                    proj_fc(tt, fc, ht)
            transp_v(tt)
            # the next tile's k/v projections (fc4, fc5) ride as PE filler
            # inside this tile's attention: they only need the prefetched
            # hidden tile, and attention's exp-bound stretches absorb them
            fillers = {}
            if tt == 0:
                fillers[0] = lambda: proj_fc(0, 2, ht, dve_copy=True)
                fillers[1] = lambda: proj_fc(0, 3, ht, dve_copy=True)
            if tt < NTT - 1:
                nxt, pf = tt + 1, prefetched
                fillers[2] = lambda n=nxt, p=pf: proj_fc(n, 4, p,
                                                         dve_copy=True)
                fillers[3] = lambda n=nxt, p=pf: proj_fc(n, 5, p,
                                                         dve_copy=True)
            final_pr0 = {}
            if tt == NTT - 1:
                def drain_filler():
                    final_pr0[0] = o_proj_final_pr0(0)
                attn_B(tt, fillers=fillers, drain_filler=drain_filler)
            else:
                attn_B(tt, fillers=fillers)
        for t4 in range(4):
            o_proj_chunk(NQT - 1, t4, final=True,
                         pre_ops=final_pr0.get(t4))

    if legalize:
        _legalize_waits(nc)
    _CACHE[key] = nc
    return nc


def _prep_inputs(hidden_states, w_q, w_k, w_v, w_o, conv_w):
    """Build the 8 per-core input maps (host-side shard + fp8 hi/lo split)."""
    f8 = ml_dtypes.float8_e4m3

    def hpairs(x):  # [2048 d, 2048 t] -> [128, 4, 8, 2, 512]
        return np.ascontiguousarray(
            x.reshape(NPR, 2, P, NTT, 512).transpose(2, 3, 0, 1, 4))

    def wpairs(x):  # [2048, 768] -> [128, 6, 8, 2, 128]
        return np.ascontiguousarray(
            x.reshape(NPR, 2, P, NF, P).transpose(2, 3, 0, 1, 4))

    def split8(x):
        hi = x.astype(f8)
        lo = (x - hi.astype(np.float32)).astype(f8)
        return hi, lo

    # hidden split is shared by the 4 cores of a batch
    h_packs = []
    for b in range(2):
        hT = np.ascontiguousarray(hidden_states[b].T)
        hi, lo = split8(hT)
        h_packs.append(np.ascontiguousarray(
            np.stack([hpairs(hi), hpairs(lo)], axis=3)))

    in_maps = []
    for c in range(8):
        b, g = c // 4, c % 4
        wqkv = np.concatenate(
            [w_q[:, g * 512:(g + 1) * 512],
             w_k[:, g * 128:(g + 1) * 128],
             w_v[:, g * 128:(g + 1) * 128]], axis=1) * WSCALE
        w_hi, w_lo = split8(wqkv)
        w_pack = np.ascontiguousarray(
            np.stack([wpairs(w_hi), wpairs(w_lo)], axis=3))
        wop = np.ascontiguousarray(w_o[g * 512:(g + 1) * 512, :]) * WSCALE
        wo_hi, wo_lo = split8(wop)
        wo_pack = np.ascontiguousarray(np.stack(
            [wo_hi.reshape(2, 2, P, D).transpose(2, 0, 1, 3),
             wo_lo.reshape(2, 2, P, D).transpose(2, 0, 1, 3)], axis=1))
        cwc = np.concatenate(
            [conv_w[g * 512:(g + 1) * 512],
             conv_w[2048 + g * 128: 2048 + (g + 1) * 128],
             conv_w[2560 + g * 128: 2560 + (g + 1) * 128]], axis=0)  # [768,4]
        cwp = np.ascontiguousarray(
            cwc.reshape(NF, P, 4).transpose(1, 0, 2).reshape(P, NF * 4)
        ).astype(np.float32)
        # residual fold: out = x + sum_k x_k w_k == sum taps with w3 += 1
        cwp[:, 3::4] += 1.0
        in_maps.append({
            "h": h_packs[b],
            "w": w_pack,
            "wo": wo_pack,
            "conv_w": cwp,
        })
    return in_maps


def kernel(hidden_states, w_q, w_k, w_v, w_o, conv_w, _trace=False):
    nc = _build()
    in_maps = _prep_inputs(
        np.asarray(hidden_states, dtype=np.float32),
        np.asarray(w_q, dtype=np.float32),
        np.asarray(w_k, dtype=np.float32),
        np.asarray(w_v, dtype=np.float32),
        np.asarray(w_o, dtype=np.float32),
        np.asarray(conv_w, dtype=np.float32),
    )
    res = run_bass_kernel_spmd(nc, in_maps, core_ids=list(range(8)),
                               trace=_trace)
    outs = [r["out"] for r in res.results]
    full = np.empty((2, S, D), dtype=np.float32)
    for b in range(2):
        acc = (outs[4 * b].astype(np.float32)
               + outs[4 * b + 1].astype(np.float32)
               + outs[4 * b + 2].astype(np.float32)
               + outs[4 * b + 3].astype(np.float32))
        full[b] = acc * (1.0 / OUT_DIV)
    if _trace:
        kernel.last_results = res
    return full


# revision 97
# speedup vs baseline: 1.0006x; 1.0006x over previous
"""CanonCausalMultiheadAttn Trainium2 kernel (fp8 DoubleRow version).

Sharding: 8 cores = 2 (batch) x 4 (kv-head groups). Core c handles batch
c//4 and kv-group g=c%4 (q heads 4g..4g+3, kv head g). w_q/w_k/w_v are
column-sharded by head group, w_o row-sharded; each core emits a partial
[S, D] output (bf16) which the host sums over the 4 groups of its batch.

The four heads of each q-tile are software-pipelined into one flat
(head, k-tile) sequence with a deep (LAG=14) pending-probs queue, so the
in-order PE queue never stalls on the exp (ACT) -> causal-mask (Pool)
producer chain; the previous q-tile's output projection is interleaved as
PE filler. All DMAs serialize through one shared HWDGE (~625ns dispatch
each), so hi/lo fp8 planes are PACKED into single DRAM tensors and output
tiles ship as one wide DMA per 128-token chunk. A short chain of warm-up
matmuls primes the PE p-state ramp (half clock until 3us continuously
busy) while the first input DMAs are in flight.

Per-core dataflow (transposed [feature, token] layout; v transposed on PE):
  qkvT[f, t] = w_qkv[:, f].T @ hT[:, t]   -- fp8e4m3 DoubleRow matmuls with
      3-term hi/lo compensation (w_hi.h_hi + w_hi.h_lo + w_lo.h_hi), pr-major
      so each hidden chunk is fully consumed as it lands.
  conv: depthwise causal taps in bf16 on DVE (tensor_scalar products,
      tensor_tensor sums); conv weights stay f32.
  scores.T[k, q] = kT.T @ qT (bf16) -> exp on ACT (scale folds the fp8
      pre-scales; bias -2ln2 keeps fp16 column sums in range)
  causal: k-tiles with k0 <= q_end only; diagonal tiles masked in-place
      by an affine_select on the (otherwise idle) Pool engine.
  attT[dh, q] += v_nat[k,:].T @ probsT  (bf16)
  colsum via DVE adds in fp16 (2x DVE mode), partition-reduced by one
      fp16 ones-matmul (ones=4.0 folds the attT scale correction).
  attT stored as fp8 hi+lo; out[t, d] = attT.T @ w_o_rows via 3-term DR,
      shipped bf16 (divided by 1024x net scale on the host).

Scales: w_qkv and w_o are pre-scaled x64 into fp8 (e4m3 = IEEE variant,
  max finite 240); hidden stays x1. qkv = 64x, scores = 4096 s (folded
  into exp scale), probs = p/4 (exp bias), att = 16*Sum p v, colsum = p/4
  summed, ones=4.0 => attT = 16*attended, out = 1024*true. Host divides.
"""

import numpy as np
import ml_dtypes
from collections import deque
from contextlib import ExitStack

import concourse.bass as bass
import concourse.tile as tile
import concourse.mybir as mybir
from concourse.bass import ds, ts
from concourse.bass_utils import run_bass_kernel_spmd
from concourse.masks import make_identity

BF16 = mybir.dt.bfloat16
F16 = mybir.dt.float16
F32 = mybir.dt.float32
FP8 = mybir.dt.float8e4
DR = mybir.MatmulPerfMode.DoubleRow
P = 128
S = 2048          # sequence length
D = 2048          # d_model
NF = 6            # feature chunks of 128: 4 q heads, 1 k, 1 v
NPR = 8           # DR contraction pairs over d_model (2048 = 8*256)
NQT = S // 512    # 4 query tiles of 512
NTT = S // 512    # 4 token tiles of 512
WSCALE = 64.0     # fp8 pre-scale on w_qkv and w_o (e4m3 max is 240)
ISQ = 1.0 / np.sqrt(128.0)
EXP_SCALE = ISQ / (WSCALE * WSCALE)   # scores PSUM holds 4096*s
EXP_BIAS = float(-2.0 * np.log(2.0))  # probs = p/4 (fp16 colsum headroom)
OUT_DIV = 1024.0  # 16 (attT) * 64 (w_o)
WARM = 7          # PE p-state warm-up matmuls
MULT = mybir.AluOpType.mult
ADD = mybir.AluOpType.add

_CACHE = {}


def _legalize_waits(nc):
    """Split multi-wait sync_info into preceding single-wait engine NOPs.

    The walrus codegen in this container accepts at most ONE sync wait per
    TPB instruction ("Too many sync wait commands"), but the Tile scheduler
    freely emits several. An engine executes its queue in order, so hoisting
    the extra waits onto NoOps right before the instruction is equivalent.
    """
    n = 0
    for f in nc.m.functions:
        for blk in f.blocks:
            out = []
            changed = False
            for inst in blk.instructions:
                si = inst.sync_info
                if (si is not None and si.on_wait and len(si.on_wait) > 1
                        and str(inst.engine) != "EngineType.Unassigned"):
                    waits = list(si.on_wait)
                    for w in waits[:-1]:
                        out.append(mybir.InstNoOp(
                            name=f"I-wf{n}", engine=inst.engine, ins=[],
                            outs=[],
                            sync_info=mybir.SyncInfo(on_wait=[w],
                                                     on_update=[])))
                        n += 1
                    si.on_wait = [waits[-1]]
                    changed = True
                out.append(inst)
            if changed:
                blk.instructions = out
    return n


def _build(legalize=True):
    key = "nc" if legalize else "nc_raw"
    if key in _CACHE:
        return _CACHE[key]
    nc = bass.Bass("TRN2", target_bir_lowering=False, debug=False)

    # hi/lo fp8 planes packed into single DRAM tensors: every DMA dispatch
    # serializes through one shared HWDGE (~625ns), so fewer+wider wins
    h_d = nc.dram_tensor("h", [P, NTT, NPR, 2, 2, 512], FP8,
                         kind="ExternalInput").ap()
    w_d = nc.dram_tensor("w", [P, NF, NPR, 2, 2, P], FP8,
                         kind="ExternalInput").ap()
    wo_d = nc.dram_tensor("wo", [P, 2, 2, 2, D], FP8,
                          kind="ExternalInput").ap()
    cw_d = nc.dram_tensor("conv_w", [P, NF * 4], F32,
                          kind="ExternalInput").ap()
    out_d = nc.dram_tensor("out", [S, D], BF16, kind="ExternalOutput").ap()

    out_v = out_d.rearrange("(po pi) d -> pi po d", pi=P)      # [128,16,2048]

    with tile.TileContext(nc) as tc, ExitStack() as ctx:
        const = ctx.enter_context(tc.tile_pool(name="const", bufs=1))
        p_ht = ctx.enter_context(tc.tile_pool(name="ht", bufs=2))
        p_work = ctx.enter_context(tc.tile_pool(name="work", bufs=3))
        p_probs = ctx.enter_context(tc.tile_pool(name="probs", bufs=6))
        p_out = ctx.enter_context(tc.tile_pool(name="outp", bufs=4))
        ps2 = ctx.enter_context(tc.tile_pool(name="ps2", bufs=2, space="PSUM"))
        ps_s = ctx.enter_context(tc.tile_pool(name="ps_s", bufs=3,
                                              space="PSUM"))
        ps3 = ctx.enter_context(tc.tile_pool(name="ps3", bufs=2, space="PSUM"))
        ps1 = ctx.enter_context(tc.tile_pool(name="ps1", bufs=1, space="PSUM"))

        # --- constants / persistent tensors ---
        # tiny warm operand memset FIRST on Pool (~100ns) so the PE p-state
        # warm-up starts ~1us before make_identity would allow
        wtiny = const.tile([P, P], BF16, tag="wtiny")
        nc.gpsimd.memset(wtiny, 0.5)
        ident = const.tile([P, P], BF16, tag="ident")
        make_identity(nc, ident)
        wscr = const.tile([P, 512], BF16, tag="wscr")
        nc.vector.memset(wscr, 0.5)
        # PE p-state warm-up: the tensor engine runs at half clock until it
        # has been continuously busy 3us; burn that ramp on dummies while
        # the first input DMAs are still in flight.
        wps = ps_s.tile([P, 512], F32, tag="s", name="warm")
        for _ in range(8):
            nc.tensor.matmul(wps[:, 0:P], lhsT=wtiny, rhs=wtiny,
                             start=True, stop=True)
        for _ in range(WARM):
            nc.tensor.matmul(wps, lhsT=wtiny, rhs=wscr, start=True, stop=True)
        cw0 = const.tile([P, NF * 4], F32, tag="cw0")
        cw = const.tile([P, NF * 4], F32, tag="cw")
        wq = const.tile([P, NF, NPR, 2, 2, P], FP8, tag="wq")
        wo = const.tile([P, 2, 2, 2, D], FP8, tag="wo")
        # raw (pre-conv) qkv.T in bf16 (64x scale), 3 leading zero columns so
        # the causal conv taps can read t-3..t-1 without edge cases
        qkvf = const.tile([P, NF, S + 3], BF16, tag="qkvf")
        nc.gpsimd.memset(qkvf[:, :, 0:3], 0.0)
        qkvb = const.tile([P, NF, S], BF16, tag="qkvb")    # conv'd qkv.T
        vnat = const.tile([P, 16, P], BF16, tag="vnat")    # v in [token, dh]
        atth = const.tile([P, 4, S], FP8, tag="atth")      # attT hi per head
        attl = const.tile([P, 4, S], FP8, tag="attl")      # attT lo per head
        ones2 = const.tile([P, P], F16, tag="ones2")
        nc.vector.memset(ones2, 4.0)
        ebias = const.tile([P, 1], F32, tag="ebias")
        nc.vector.memset(ebias, EXP_BIAS)

        obuf = {}  # t4 -> wide bf16 output tile (one DMA per token chunk)

        def oproj_trio(tt16, op, dt, pr_, k0):
            # atth terms first: the attl piece lands one DVE op later in
            # the finalize chain, so leading with atth starts ~190ns sooner
            for k, (lhs, hl) in enumerate(
                    ((atth, 0), (atth, 1), (attl, 0))):
                nc.tensor.matmul(
                    op,
                    lhsT=lhs[:, ds(2 * pr_, 2), ds(tt16 * P, P)],
                    rhs=wo[:, hl, pr_, :, ds(dt * 512, 512)],
                    start=(k0 + k == 0), stop=(k0 + k == 5),
                    perf_mode=DR)

        def o_proj_final_pr0(t4):
            # head-0/1 trios of a final chunk: eligible as soon as the
            # first head pair is normalized, used as PE filler while the
            # last head's denominator drains on DVE
            tt16 = (NQT - 1) * 4 + t4
            ops = {}
            for dt in range(4):
                if dt % 2 == 1:
                    ops[dt] = ps_s.tile([P, 512], F32, tag="s", name="op")
                else:
                    ops[dt] = ps2.tile([P, 512], F32, tag="proj", name="op")
                oproj_trio(tt16, ops[dt], dt, 0, 0)
            return ops

        def o_proj_chunk(qt, t4, final=False, dts=(0, 1, 2, 3),
                         pre_ops=None):
            # output projection for one token-128-tile of q-tile qt; the
            # PSUM->SBUF copies land in one wide bf16 tile which ships as a
            # SINGLE DMA per token chunk (HWDGE dispatch is the scarce
            # resource, not DMA bandwidth)
            tt16 = qt * 4 + t4
            if t4 not in obuf:
                obuf[t4] = p_out.tile([P, 4, 512], BF16, tag="ob", name="ob")
            ob = obuf[t4]

            def trio(op, dt, pr_, k0):
                oproj_trio(tt16, op, dt, pr_, k0)

            for dt in dts:
                if pre_ops is not None:
                    op = pre_ops[dt]
                else:
                    if final and dt % 2 == 1:
                        op = ps_s.tile([P, 512], F32, tag="s", name="op")
                    else:
                        op = ps2.tile([P, 512], F32, tag="proj", name="op")
                    trio(op, dt, 0, 0)
                trio(op, dt, 1, 3)
                if final:
                    # alternate engines so the last chunk's copies overlap;
                    # ship in pieces so earlier DMAs overlap later matmuls
                    if dt == 3 and t4 == 3:
                        # the very last copy splits across ACT+DVE halves
                        # running in parallel (~390ns instead of 612)
                        nc.scalar.copy(ob[:, dt, ds(0, 256)],
                                       op[:, ds(0, 256)])
                        nc.vector.tensor_copy(ob[:, dt, ds(256, 256)],
                                              op[:, ds(256, 256)])
                    elif dt % 2 == 0:
                        nc.vector.tensor_copy(ob[:, dt, :], op)
                    else:
                        nc.scalar.copy(ob[:, dt, :], op)
                    if dt == 1:
                        nc.sync.dma_start(out_v[:, tt16, ds(0, 1024)],
                                          ob[:, ds(0, 2)])
                    elif dt == 2:
                        nc.sync.dma_start(out_v[:, tt16, ds(1024, 512)],
                                          ob[:, ds(2, 1)])
                    elif dt == 3:
                        nc.sync.dma_start(out_v[:, tt16, ds(1536, 512)],
                                          ob[:, ds(3, 1)])
                elif dt == 3:
                    nc.scalar.copy(ob[:, dt, :], op)
                else:
                    nc.vector.tensor_copy(ob[:, dt, :], op)
            if dts[-1] == 3:
                if not final:
                    nc.sync.dma_start(out_v[:, tt16, :], ob)
                del obuf[t4]

        def attn_B(qt, fillers=None, drain_filler=None):
            # attention for q-tile qt (needs token tiles <= qt). The four
            # heads are software-pipelined into ONE flat (h, kt) sequence:
            # scores of head h+1 are emitted while head h's attended matmuls
            # drain, so the in-order PE queue never stalls on the
            # exp (ACT) -> mask (Pool) producer chain. The previous q-tile's
            # output projection is interleaved as additional PE filler.
            nk = 4 * (qt + 1)
            LAG = 14
            state = {}  # h -> (att, colsum)
            fin = {}    # h -> (att, colsum) awaiting denominator finalize
            fin_q = deque()  # [h, consumes-since-ready]
            pend = deque()
            pr_quad = None

            def consume():
                ch, ppr, px0, pkt = pend.popleft()
                att, colsum = state[ch] if ch in state else fin[ch]
                nc.tensor.matmul(
                    att[:, px0:512], lhsT=vnat[:, pkt, :],
                    rhs=ppr[:, px0:512],
                    start=(pkt == 0), stop=(pkt == nk - 1))
                # softmax denominator: accumulate exp'd probs on DVE
                # (partition dim reduced by ONE ones-matmul at the end)
                if pkt == 0:
                    nc.vector.tensor_copy(colsum, ppr)
                else:
                    nc.vector.tensor_add(
                        colsum[:, px0:512], colsum[:, px0:512],
                        ppr[:, px0:512])
                if pkt == nk - 1:
                    fin[ch] = state.pop(ch)
                    fin_q.append([ch, 0])

            def finalize(ch):
                att, colsum = fin[ch]
                smp = ps1.tile([P, 512], F32, tag="small")
                rec = p_work.tile([P, 512], F32, tag="rec")
                t16 = p_work.tile([P, 512], F16, tag="t16")
                if qt == NQT - 1 and ch >= 2:
                    # last heads before the final output projection: the
                    # whole normalization chain runs in 128-col pieces
                    # (ones-matmul and reciprocal included) so the first
                    # final o_proj trios start ~0.6us sooner
                    for pc in range(4):
                        c = ds(pc * P, P)
                        nc.tensor.matmul(smp[:, c], lhsT=ones2,
                                         rhs=colsum[:, c],
                                         start=True, stop=True)
                        nc.vector.reciprocal(rec[:, c], smp[:, c])
                        nc.vector.tensor_mul(t16[:, c], att[:, c], rec[:, c])
                        nc.vector.tensor_copy(
                            atth[:, ch, ds(qt * 512 + pc * P, P)], t16[:, c])
                        nc.vector.tensor_sub(
                            attl[:, ch, ds(qt * 512 + pc * P, P)], t16[:, c],
                            atth[:, ch, ds(qt * 512 + pc * P, P)])
                else:
                    nc.tensor.matmul(smp, lhsT=ones2, rhs=colsum,
                                     start=True, stop=True)
                    nc.vector.reciprocal(rec, smp)
                    nc.vector.tensor_mul(t16, att, rec)
                    nc.gpsimd.tensor_copy(atth[:, ch, ts(qt, 512)], t16)
                    nc.gpsimd.tensor_sub(attl[:, ch, ts(qt, 512)], t16,
                                         atth[:, ch, ts(qt, 512)])
                del fin[ch]

            for h in range(4):
                if fillers and h in fillers:
                    fillers[h]()
                state[h] = (ps3.tile([P, 512], F32, tag="att", name="att"),
                            p_work.tile([P, 512], F16, tag="colsum",
                                        name="colsum"))
                for kt in range(nk):
                    # shorter pending queue only near the END of the very
                    # last head: early kts keep the deep pipeline (PE ahead
                    # of exp), the tail still drains early so the final
                    # output projection starts with less latency
                    lag = 4 if (qt == NQT - 1 and h == 3
                                and kt >= nk - 6) else LAG
                    # previous q-tile's output projection emitted mid-head;
                    # on long tiles spread it at four points so the filler
                    # matches the exp-bound score/attend cadence
                    if qt > 0:
                        pts = {8: (2, 4, 6, 7), 12: (3, 6, 9, 11),
                               16: (4, 8, 11, 14)}[nk]
                        if kt in pts:
                            o_proj_chunk(qt - 1, h, dts=(pts.index(kt),))
                    j = kt - 4 * qt
                    x0 = j * P if j >= 0 else 0
                    F = 512 - x0
                    sp = ps_s.tile([P, 512], F32, tag="s")
                    nc.tensor.matmul(
                        sp[:, x0:512],
                        lhsT=qkvb[:, 4, ds(kt * P, P)],
                        rhs=qkvb[:, h, ds(qt * 512 + x0, F)],
                        start=True, stop=True,
                    )
                    if kt % 4 == 0:
                        pr_quad = p_probs.tile([P, 4, 512], BF16, tag="probs")
                    pr = pr_quad[:, kt % 4, :]
                    nc.scalar.activation(
                        pr[:, x0:512], sp[:, x0:512],
                        mybir.ActivationFunctionType.Exp,
                        scale=EXP_SCALE, bias=ebias)
                    if j >= 0:
                        # zero the k>q half of the diagonal tile in place
                        # (local col c vs partition p: keep iff c >= p)
                        nc.gpsimd.affine_select(
                            out=pr[:, x0:512], in_=pr[:, x0:512],
                            pattern=[[1, F]], base=0,
                            channel_multiplier=-1,
                            compare_op=mybir.AluOpType.is_ge, fill=0.0)
                    pend.append((h, pr, x0, kt))
                    thr = 1 if qt == NQT - 1 else 3
                    while len(pend) > lag:
                        consume()
                        for e in fin_q:
                            e[1] += 1
                        if fin_q and fin_q[0][1] >= thr:
                            finalize(fin_q.popleft()[0])
            while pend:
                consume()
            if drain_filler is not None:
                # PE work emitted ahead of the remaining finalizes, whose
                # ones-matmuls stall in-order on the DVE colsum chain
                drain_filler()
            while fin_q:
                finalize(fin_q.popleft()[0])

        # ------- Fused phases: per token tile: projection+conv, then the
        # attention q-tile that just became computable, then the (pipelined)
        # output projection of the previous q-tile.
        def ht_alloc():
            return p_ht.tile([P, NPR, 2, 2, 512], FP8, tag="ht", name="ht")

        def ht_dispatch(ht, tt, chunks):
            for c0, w_ in chunks:
                nc.sync.dma_start(ht[:, ds(c0, w_)],
                                  h_d[:, tt, ds(c0, w_)])

        def conv_fc(tt, fc, pp, dve_copy=False):
            t0 = tt * 512
            # pre-conv x (64x) -> bf16 for the DVE conv taps. When run
            # as attention filler, the copy goes on DVE because ACT is
            # clogged with exp tiles there.
            if dve_copy:
                nc.vector.tensor_copy(qkvf[:, fc, ds(3 + t0, 512)], pp)
            else:
                nc.scalar.copy(qkvf[:, fc, ds(3 + t0, 512)], pp)
            # conv taps: out[t] = x[t] + sum_k x[t+k-3]*w[k].
            # Products via tensor_scalar (4x DVE mode — the tensor-tensor
            # variant gets no fast mode), sums via bf16 tensor_tensor (2x)
            ca = p_work.tile([P, 512], BF16, tag="ctmpa", name="ca")
            cb = p_work.tile([P, 512], BF16, tag="ctmpb", name="cb")
            nc.vector.tensor_scalar(
                ca, qkvf[:, fc, ds(t0 + 0, 512)],
                cw[:, fc * 4 + 0: fc * 4 + 1], None, op0=MULT)
            nc.vector.tensor_scalar(
                cb, qkvf[:, fc, ds(t0 + 1, 512)],
                cw[:, fc * 4 + 1: fc * 4 + 2], None, op0=MULT)
            nc.vector.tensor_add(ca, ca, cb)
            nc.vector.tensor_scalar(
                cb, qkvf[:, fc, ds(t0 + 2, 512)],
                cw[:, fc * 4 + 2: fc * 4 + 3], None, op0=MULT)
            nc.vector.tensor_add(ca, ca, cb)
            # last tap's weight is pre-biased +1 on the host, folding the
            # residual x[t] into the same tensor_scalar product
            nc.vector.tensor_scalar(
                cb, qkvf[:, fc, ds(t0 + 3, 512)],
                cw[:, fc * 4 + 3: fc * 4 + 4], None, op0=MULT)
            nc.vector.tensor_add(qkvb[:, fc, ts(tt, 512)], ca, cb)

        def proj_fc(tt, fc, ht, dve_copy=False):
            pp = ps2.tile([P, 512], F32, tag="proj", name="pp")
            # pr-major: each hidden pr chunk is fully consumed (all three
            # hi/lo terms) as soon as it lands, minimizing startup stalls
            k = 0
            for pr_ in range(NPR):
                for whl, hhl in ((0, 0), (0, 1), (1, 0)):
                    nc.tensor.matmul(
                        pp, lhsT=wq[:, fc, pr_, whl], rhs=ht[:, pr_, hhl],
                        start=(k == 0), stop=(k == 3 * NPR - 1),
                        perf_mode=DR)
                    k += 1
            conv_fc(tt, fc, pp, dve_copy)

        def transp_v(tt):
            # v (fc=5) of this token tile -> natural [token, dh] layout
            trp = ps1.tile([P, 512], BF16, tag="small")
            for j in range(4):
                nc.tensor.transpose(trp[:, ds(j * P, P)],
                                    qkvb[:, 5, ds((tt * 4 + j) * P, P)],
                                    ident)
            nc.vector.tensor_copy(vnat[:, ds(tt * 4, 4), :], trp)

        prefetched = None
        for tt in range(NTT):
            if tt == 0:
                # The DMA engine pool executes one transfer at a time, so
                # the dispatch sequence below is a global priority schedule:
                # interleave hidden pr-chunks with the weight packs in
                # exactly the order the paired pr-major projection consumes
                # them (fc4+fc5 first, pr chunk by pr chunk).
                ht = ht_alloc()
                ht_dispatch(ht, 0, ((0, 1),))
                nc.sync.dma_start(wq[:, 4, ds(0, 4)], w_d[:, 4, ds(0, 4)])
                nc.sync.dma_start(wq[:, 5, ds(0, 4)], w_d[:, 5, ds(0, 4)])
                ht_dispatch(ht, 0, ((1, 1), (2, 2)))
                nc.sync.dma_start(wq[:, 4, ds(4, 4)], w_d[:, 4, ds(4, 4)])
                nc.sync.dma_start(wq[:, 5, ds(4, 4)], w_d[:, 5, ds(4, 4)])
                ht_dispatch(ht, 0, ((4, 2), (6, 2)))
                nc.sync.dma_start(cw0, cw_d)
                nc.vector.tensor_copy(cw, cw0)
                for fc in (0, 1, 2, 3):
                    nc.sync.dma_start(wq[:, fc], w_d[:, fc])
                prefetched = ht_alloc()
                ht_dispatch(prefetched, 1, ((0, 4), (4, 4)))
            else:
                ht = prefetched
                if tt == 1:
                    nc.sync.dma_start(wo, wo_d)
                if tt < NTT - 1:
                    prefetched = ht_alloc()
                    ht_dispatch(prefetched, tt + 1, ((0, 4), (4, 4)))

            if tt == 0:
                # fc4+fc5 chains interleaved pr-major: two PSUM chains
                # consume each hidden pr chunk at the DMA supply rate; the
                # remaining projections ride inside the attention blocks
                pps = {4: ps2.tile([P, 512], F32, tag="proj", name="pp"),
                       5: ps2.tile([P, 512], F32, tag="proj", name="pp")}
                ks = {4: 0, 5: 0}
                for pr_ in range(NPR):
                    for fc in (4, 5):
                        for whl, hhl in ((0, 0), (0, 1), (1, 0)):
                            nc.tensor.matmul(
                                pps[fc], lhsT=wq[:, fc, pr_, whl],
                                rhs=ht[:, pr_, hhl],
                                start=(ks[fc] == 0),
                                stop=(ks[fc] == 3 * NPR - 1), perf_mode=DR)
                            ks[fc] += 1
                    # pad known DMA-supply stalls with warm matmuls so the
                    # PE p-state ramp survives the bandwidth-gated stretch
                    for _ in range({1: 2, 3: 4}.get(pr_, 0)):
                        wpad = ps_s.tile([P, 512], F32, tag="s", name="wpad")
                        nc.tensor.matmul(wpad, lhsT=wtiny, rhs=wscr,
                                         start=True, stop=True)
                conv_fc(0, 4, pps[4])
                conv_fc(0, 5, pps[5])
                proj_fc(0, 0, ht)
                proj_fc(0, 1, ht)
            else:
                for fc in (0, 1, 2, 3):
                    proj_fc(tt, fc, ht)
            transp_v(tt)
            # the next tile's k/v projections (fc4, fc5) ride as PE filler
            # inside this tile's attention: they only need the prefetched
            # hidden tile, and attention's exp-bound stretches absorb them
            fillers = {}
            if tt == 0:
                fillers[0] = lambda: proj_fc(0, 2, ht, dve_copy=True)
                fillers[1] = lambda: proj_fc(0, 3, ht, dve_copy=True)
            if tt < NTT - 1:
                nxt, pf = tt + 1, prefetched
                fillers[2] = lambda n=nxt, p=pf: proj_fc(n, 4, p,
                                                         dve_copy=True)
                fillers[3] = lambda n=nxt, p=pf: proj_fc(n, 5, p,
                                                         dve_copy=True)
            final_pr0 = {}
            if tt == NTT - 1:
                def drain_filler():
                    final_pr0[0] = o_proj_final_pr0(0)
                attn_B(tt, fillers=fillers, drain_filler=drain_filler)
            else:
                attn_B(tt, fillers=fillers)
        for t4 in range(4):
            o_proj_chunk(NQT - 1, t4, final=True,
                         pre_ops=final_pr0.get(t4))

    if legalize:
        _legalize_waits(nc)
    _CACHE[key] = nc
    return nc


def _prep_inputs(hidden_states, w_q, w_k, w_v, w_o, conv_w):
    """Build the 8 per-core input maps (host-side shard + fp8 hi/lo split)."""
    f8 = ml_dtypes.float8_e4m3

    def hpairs(x):  # [2048 d, 2048 t] -> [128, 4, 8, 2, 512]
        return np.ascontiguousarray(
            x.reshape(NPR, 2, P, NTT, 512).transpose(2, 3, 0, 1, 4))

    def wpairs(x):  # [2048, 768] -> [128, 6, 8, 2, 128]
        return np.ascontiguousarray(
            x.reshape(NPR, 2, P, NF, P).transpose(2, 3, 0, 1, 4))

    def split8(x):
        hi = x.astype(f8)
        lo = (x - hi.astype(np.float32)).astype(f8)
        return hi, lo

    # hidden split is shared by the 4 cores of a batch
    h_packs = []
    for b in range(2):
        hT = np.ascontiguousarray(hidden_states[b].T)
        hi, lo = split8(hT)
        h_packs.append(np.ascontiguousarray(
            np.stack([hpairs(hi), hpairs(lo)], axis=3)))

    in_maps = []
    for c in range(8):
        b, g = c // 4, c % 4
        wqkv = np.concatenate(
            [w_q[:, g * 512:(g + 1) * 512],
             w_k[:, g * 128:(g + 1) * 128],
             w_v[:, g * 128:(g + 1) * 128]], axis=1) * WSCALE
        w_hi, w_lo = split8(wqkv)
        w_pack = np.ascontiguousarray(
            np.stack([wpairs(w_hi), wpairs(w_lo)], axis=3))
        wop = np.ascontiguousarray(w_o[g * 512:(g + 1) * 512, :]) * WSCALE
        wo_hi, wo_lo = split8(wop)
        wo_pack = np.ascontiguousarray(np.stack(
            [wo_hi.reshape(2, 2, P, D).transpose(2, 0, 1, 3),
             wo_lo.reshape(2, 2, P, D).transpose(2, 0, 1, 3)], axis=1))
        cwc = np.concatenate(
            [conv_w[g * 512:(g + 1) * 512],
             conv_w[2048 + g * 128: 2048 + (g + 1) * 128],
             conv_w[2560 + g * 128: 2560 + (g + 1) * 128]], axis=0)  # [768,4]
        cwp = np.ascontiguousarray(
            cwc.reshape(NF, P, 4).transpose(1, 0, 2).reshape(P, NF * 4)
        ).astype(np.float32)
        # residual fold: out = x + sum_k x_k w_k == sum taps with w3 += 1
        cwp[:, 3::4] += 1.0
        in_maps.append({
            "h": h_packs[b],
            "w": w_pack,
            "wo": wo_pack,
            "conv_w": cwp,
        })
    return in_maps


def kernel(hidden_states, w_q, w_k, w_v, w_o, conv_w, _trace=False):
    nc = _build()
    in_maps = _prep_inputs(
        np.asarray(hidden_states, dtype=np.float32),
        np.asarray(w_q, dtype=np.float32),
        np.asarray(w_k, dtype=np.float32),
        np.asarray(w_v, dtype=np.float32),
        np.asarray(w_o, dtype=np.float32),
        np.asarray(conv_w, dtype=np.float32),
    )
    res = run_bass_kernel_spmd(nc, in_maps, core_ids=list(range(8)),
                               trace=_trace)
    outs = [r["out"] for r in res.results]
    full = np.empty((2, S, D), dtype=np.float32)
    for b in range(2):
        acc = (outs[4 * b].astype(np.float32)
               + outs[4 * b + 1].astype(np.float32)
               + outs[4 * b + 2].astype(np.float32)
               + outs[4 * b + 3].astype(np.float32))
        full[b] = acc * (1.0 / OUT_DIV)
    if _trace:
        kernel.last_results = res
    return full


# revision 98
# speedup vs baseline: 1.0008x; 1.0002x over previous
"""CanonCausalMultiheadAttn Trainium2 kernel (fp8 DoubleRow version).

Sharding: 8 cores = 2 (batch) x 4 (kv-head groups). Core c handles batch
c//4 and kv-group g=c%4 (q heads 4g..4g+3, kv head g). w_q/w_k/w_v are
column-sharded by head group, w_o row-sharded; each core emits a partial
[S, D] output (bf16) which the host sums over the 4 groups of its batch.

The four heads of each q-tile are software-pipelined into one flat
(head, k-tile) sequence with a deep (LAG=14) pending-probs queue, so the
in-order PE queue never stalls on the exp (ACT) -> causal-mask (Pool)
producer chain; the previous q-tile's output projection is interleaved as
PE filler. All DMAs serialize through one shared HWDGE (~625ns dispatch
each), so hi/lo fp8 planes are PACKED into single DRAM tensors and output
tiles ship as one wide DMA per 128-token chunk. A short chain of warm-up
matmuls primes the PE p-state ramp (half clock until 3us continuously
busy) while the first input DMAs are in flight.

Per-core dataflow (transposed [feature, token] layout; v transposed on PE):
  qkvT[f, t] = w_qkv[:, f].T @ hT[:, t]   -- fp8e4m3 DoubleRow matmuls with
      3-term hi/lo compensation (w_hi.h_hi + w_hi.h_lo + w_lo.h_hi), pr-major
      so each hidden chunk is fully consumed as it lands.
  conv: depthwise causal taps in bf16 on DVE (tensor_scalar products,
      tensor_tensor sums); conv weights stay f32.
  scores.T[k, q] = kT.T @ qT (bf16) -> exp on ACT (scale folds the fp8
      pre-scales; bias -2ln2 keeps fp16 column sums in range)
  causal: k-tiles with k0 <= q_end only; diagonal tiles masked in-place
      by an affine_select on the (otherwise idle) Pool engine.
  attT[dh, q] += v_nat[k,:].T @ probsT  (bf16)
  colsum via DVE adds in fp16 (2x DVE mode), partition-reduced by one
      fp16 ones-matmul (ones=4.0 folds the attT scale correction).
  attT stored as fp8 hi+lo; out[t, d] = attT.T @ w_o_rows via 3-term DR,
      shipped bf16 (divided by 1024x net scale on the host).

Scales: w_qkv and w_o are pre-scaled x64 into fp8 (e4m3 = IEEE variant,
  max finite 240); hidden stays x1. qkv = 64x, scores = 4096 s (folded
  into exp scale), probs = p/4 (exp bias), att = 16*Sum p v, colsum = p/4
  summed, ones=4.0 => attT = 16*attended, out = 1024*true. Host divides.
"""

import numpy as np
import ml_dtypes
from collections import deque
from contextlib import ExitStack

import concourse.bass as bass
import concourse.tile as tile
import concourse.mybir as mybir
from concourse.bass import ds, ts
from concourse.bass_utils import run_bass_kernel_spmd
from concourse.masks import make_identity

BF16 = mybir.dt.bfloat16
F16 = mybir.dt.float16
F32 = mybir.dt.float32
FP8 = mybir.dt.float8e4
DR = mybir.MatmulPerfMode.DoubleRow
P = 128
S = 2048          # sequence length
D = 2048          # d_model
NF = 6            # feature chunks of 128: 4 q heads, 1 k, 1 v
NPR = 8           # DR contraction pairs over d_model (2048 = 8*256)
NQT = S // 512    # 4 query tiles of 512
NTT = S // 512    # 4 token tiles of 512
WSCALE = 64.0     # fp8 pre-scale on w_qkv and w_o (e4m3 max is 240)
ISQ = 1.0 / np.sqrt(128.0)
EXP_SCALE = ISQ / (WSCALE * WSCALE)   # scores PSUM holds 4096*s
EXP_BIAS = float(-2.0 * np.log(2.0))  # probs = p/4 (fp16 colsum headroom)
OUT_DIV = 1024.0  # 16 (attT) * 64 (w_o)
WARM = 7          # PE p-state warm-up matmuls
MULT = mybir.AluOpType.mult
ADD = mybir.AluOpType.add

_CACHE = {}


def _legalize_waits(nc):
    """Split multi-wait sync_info into preceding single-wait engine NOPs.

    The walrus codegen in this container accepts at most ONE sync wait per
    TPB instruction ("Too many sync wait commands"), but the Tile scheduler
    freely emits several. An engine executes its queue in order, so hoisting
    the extra waits onto NoOps right before the instruction is equivalent.
    """
    n = 0
    for f in nc.m.functions:
        for blk in f.blocks:
            out = []
            changed = False
            for inst in blk.instructions:
                si = inst.sync_info
                if (si is not None and si.on_wait and len(si.on_wait) > 1
                        and str(inst.engine) != "EngineType.Unassigned"):
                    waits = list(si.on_wait)
                    for w in waits[:-1]:
                        out.append(mybir.InstNoOp(
                            name=f"I-wf{n}", engine=inst.engine, ins=[],
                            outs=[],
                            sync_info=mybir.SyncInfo(on_wait=[w],
                                                     on_update=[])))
                        n += 1
                    si.on_wait = [waits[-1]]
                    changed = True
                out.append(inst)
            if changed:
                blk.instructions = out
    return n


def _build(legalize=True):
    key = "nc" if legalize else "nc_raw"
    if key in _CACHE:
        return _CACHE[key]
    nc = bass.Bass("TRN2", target_bir_lowering=False, debug=False)

    # hi/lo fp8 planes packed into single DRAM tensors: every DMA dispatch
    # serializes through one shared HWDGE (~625ns), so fewer+wider wins
    h_d = nc.dram_tensor("h", [P, NTT, NPR, 2, 2, 512], FP8,
                         kind="ExternalInput").ap()
    w_d = nc.dram_tensor("w", [P, NF, NPR, 2, 2, P], FP8,
                         kind="ExternalInput").ap()
    wo_d = nc.dram_tensor("wo", [P, 2, 2, 2, D], FP8,
                          kind="ExternalInput").ap()
    cw_d = nc.dram_tensor("conv_w", [P, NF * 4], F32,
                          kind="ExternalInput").ap()
    out_d = nc.dram_tensor("out", [S, D], BF16, kind="ExternalOutput").ap()

    out_v = out_d.rearrange("(po pi) d -> pi po d", pi=P)      # [128,16,2048]

    with tile.TileContext(nc) as tc, ExitStack() as ctx:
        const = ctx.enter_context(tc.tile_pool(name="const", bufs=1))
        p_ht = ctx.enter_context(tc.tile_pool(name="ht", bufs=2))
        p_work = ctx.enter_context(tc.tile_pool(name="work", bufs=3))
        p_probs = ctx.enter_context(tc.tile_pool(name="probs", bufs=6))
        p_out = ctx.enter_context(tc.tile_pool(name="outp", bufs=4))
        ps2 = ctx.enter_context(tc.tile_pool(name="ps2", bufs=2, space="PSUM"))
        ps_s = ctx.enter_context(tc.tile_pool(name="ps_s", bufs=3,
                                              space="PSUM"))
        ps3 = ctx.enter_context(tc.tile_pool(name="ps3", bufs=2, space="PSUM"))
        ps1 = ctx.enter_context(tc.tile_pool(name="ps1", bufs=1, space="PSUM"))

        # --- constants / persistent tensors ---
        # tiny warm operand memset FIRST on Pool (~100ns) so the PE p-state
        # warm-up starts ~1us before make_identity would allow
        wtiny = const.tile([P, P], BF16, tag="wtiny")
        nc.gpsimd.memset(wtiny, 0.5)
        ident = const.tile([P, P], BF16, tag="ident")
        make_identity(nc, ident)
        wscr = const.tile([P, 512], BF16, tag="wscr")
        nc.vector.memset(wscr, 0.5)
        # PE p-state warm-up: the tensor engine runs at half clock until it
        # has been continuously busy 3us; burn that ramp on dummies while
        # the first input DMAs are still in flight.
        wps = ps_s.tile([P, 512], F32, tag="s", name="warm")
        for _ in range(8):
            nc.tensor.matmul(wps[:, 0:P], lhsT=wtiny, rhs=wtiny,
                             start=True, stop=True)
        for _ in range(WARM):
            nc.tensor.matmul(wps, lhsT=wtiny, rhs=wscr, start=True, stop=True)
        cw0 = const.tile([P, NF * 4], F32, tag="cw0")
        cw = const.tile([P, NF * 4], F32, tag="cw")
        wq = const.tile([P, NF, NPR, 2, 2, P], FP8, tag="wq")
        wo = const.tile([P, 2, 2, 2, D], FP8, tag="wo")
        # raw (pre-conv) qkv.T in bf16 (64x scale), 3 leading zero columns so
        # the causal conv taps can read t-3..t-1 without edge cases
        qkvf = const.tile([P, NF, S + 3], BF16, tag="qkvf")
        nc.gpsimd.memset(qkvf[:, :, 0:3], 0.0)
        qkvb = const.tile([P, NF, S], BF16, tag="qkvb")    # conv'd qkv.T
        vnat = const.tile([P, 16, P], BF16, tag="vnat")    # v in [token, dh]
        atth = const.tile([P, 4, S], FP8, tag="atth")      # attT hi per head
        attl = const.tile([P, 4, S], FP8, tag="attl")      # attT lo per head
        ones2 = const.tile([P, P], F16, tag="ones2")
        nc.vector.memset(ones2, 4.0)
        ebias = const.tile([P, 1], F32, tag="ebias")
        nc.vector.memset(ebias, EXP_BIAS)

        obuf = {}  # t4 -> wide bf16 output tile (one DMA per token chunk)

        def oproj_trio(tt16, op, dt, pr_, k0):
            # atth terms first: the attl piece lands one DVE op later in
            # the finalize chain, so leading with atth starts ~190ns sooner
            for k, (lhs, hl) in enumerate(
                    ((atth, 0), (atth, 1), (attl, 0))):
                nc.tensor.matmul(
                    op,
                    lhsT=lhs[:, ds(2 * pr_, 2), ds(tt16 * P, P)],
                    rhs=wo[:, hl, pr_, :, ds(dt * 512, 512)],
                    start=(k0 + k == 0), stop=(k0 + k == 5),
                    perf_mode=DR)

        def o_proj_final_pr0(t4):
            # head-0/1 trios of a final chunk: eligible as soon as the
            # first head pair is normalized, used as PE filler while the
            # last head's denominator drains on DVE
            tt16 = (NQT - 1) * 4 + t4
            ops = {}
            for dt in range(4):
                if dt % 2 == 1:
                    ops[dt] = ps_s.tile([P, 512], F32, tag="s", name="op")
                else:
                    ops[dt] = ps2.tile([P, 512], F32, tag="proj", name="op")
                oproj_trio(tt16, ops[dt], dt, 0, 0)
            return ops

        def o_proj_chunk(qt, t4, final=False, dts=(0, 1, 2, 3),
                         pre_ops=None):
            # output projection for one token-128-tile of q-tile qt; the
            # PSUM->SBUF copies land in one wide bf16 tile which ships as a
            # SINGLE DMA per token chunk (HWDGE dispatch is the scarce
            # resource, not DMA bandwidth)
            tt16 = qt * 4 + t4
            if t4 not in obuf:
                obuf[t4] = p_out.tile([P, 4, 512], BF16, tag="ob", name="ob")
            ob = obuf[t4]

            def trio(op, dt, pr_, k0):
                oproj_trio(tt16, op, dt, pr_, k0)

            for dt in dts:
                if pre_ops is not None:
                    op = pre_ops[dt]
                else:
                    if final and dt % 2 == 1:
                        op = ps_s.tile([P, 512], F32, tag="s", name="op")
                    else:
                        op = ps2.tile([P, 512], F32, tag="proj", name="op")
                    trio(op, dt, 0, 0)
                trio(op, dt, 1, 3)
                if final:
                    # alternate engines so the last chunk's copies overlap;
                    # ship in pieces so earlier DMAs overlap later matmuls
                    if dt == 3 and t4 == 3:
                        # the very last copy splits across ACT+DVE halves
                        # running in parallel (~390ns instead of 612)
                        nc.scalar.copy(ob[:, dt, ds(0, 256)],
                                       op[:, ds(0, 256)])
                        nc.vector.tensor_copy(ob[:, dt, ds(256, 256)],
                                              op[:, ds(256, 256)])
                    elif dt % 2 == 0:
                        nc.vector.tensor_copy(ob[:, dt, :], op)
                    else:
                        nc.scalar.copy(ob[:, dt, :], op)
                    if dt == 1:
                        nc.sync.dma_start(out_v[:, tt16, ds(0, 1024)],
                                          ob[:, ds(0, 2)])
                    elif dt == 2:
                        nc.sync.dma_start(out_v[:, tt16, ds(1024, 512)],
                                          ob[:, ds(2, 1)])
                    elif dt == 3:
                        nc.sync.dma_start(out_v[:, tt16, ds(1536, 512)],
                                          ob[:, ds(3, 1)])
                elif dt == 3:
                    nc.scalar.copy(ob[:, dt, :], op)
                else:
                    nc.vector.tensor_copy(ob[:, dt, :], op)
            if dts[-1] == 3:
                if not final:
                    nc.sync.dma_start(out_v[:, tt16, :], ob)
                del obuf[t4]

        def attn_B(qt, fillers=None, drain_filler=None):
            # attention for q-tile qt (needs token tiles <= qt). The four
            # heads are software-pipelined into ONE flat (h, kt) sequence:
            # scores of head h+1 are emitted while head h's attended matmuls
            # drain, so the in-order PE queue never stalls on the
            # exp (ACT) -> mask (Pool) producer chain. The previous q-tile's
            # output projection is interleaved as additional PE filler.
            nk = 4 * (qt + 1)
            LAG = 14
            state = {}  # h -> (att, colsum)
            fin = {}    # h -> (att, colsum) awaiting denominator finalize
            fin_q = deque()  # [h, consumes-since-ready]
            pend = deque()
            pr_quad = None

            def consume():
                ch, ppr, px0, pkt = pend.popleft()
                att, colsum = state[ch] if ch in state else fin[ch]
                nc.tensor.matmul(
                    att[:, px0:512], lhsT=vnat[:, pkt, :],
                    rhs=ppr[:, px0:512],
                    start=(pkt == 0), stop=(pkt == nk - 1))
                # softmax denominator: accumulate exp'd probs on DVE
                # (partition dim reduced by ONE ones-matmul at the end)
                if pkt == 0:
                    nc.vector.tensor_copy(colsum, ppr)
                else:
                    nc.vector.tensor_add(
                        colsum[:, px0:512], colsum[:, px0:512],
                        ppr[:, px0:512])
                if pkt == nk - 1:
                    fin[ch] = state.pop(ch)
                    fin_q.append([ch, 0])

            def finalize(ch):
                att, colsum = fin[ch]
                smp = ps1.tile([P, 512], F32, tag="small")
                rec = p_work.tile([P, 512], F32, tag="rec")
                t16 = p_work.tile([P, 512], F16, tag="t16")
                if qt == NQT - 1 and ch >= 2:
                    # last heads before the final output projection: the
                    # whole normalization chain runs in 128-col pieces
                    # (ones-matmul and reciprocal included) so the first
                    # final o_proj trios start ~0.6us sooner
                    for pc in range(4):
                        c = ds(pc * P, P)
                        nc.tensor.matmul(smp[:, c], lhsT=ones2,
                                         rhs=colsum[:, c],
                                         start=True, stop=True)
                        nc.vector.reciprocal(rec[:, c], smp[:, c])
                        nc.vector.tensor_mul(t16[:, c], att[:, c], rec[:, c])
                        nc.vector.tensor_copy(
                            atth[:, ch, ds(qt * 512 + pc * P, P)], t16[:, c])
                        nc.vector.tensor_sub(
                            attl[:, ch, ds(qt * 512 + pc * P, P)], t16[:, c],
                            atth[:, ch, ds(qt * 512 + pc * P, P)])
                else:
                    nc.tensor.matmul(smp, lhsT=ones2, rhs=colsum,
                                     start=True, stop=True)
                    nc.vector.reciprocal(rec, smp)
                    nc.vector.tensor_mul(t16, att, rec)
                    nc.gpsimd.tensor_copy(atth[:, ch, ts(qt, 512)], t16)
                    nc.gpsimd.tensor_sub(attl[:, ch, ts(qt, 512)], t16,
                                         atth[:, ch, ts(qt, 512)])
                del fin[ch]

            for h in range(4):
                if fillers and h in fillers:
                    fillers[h]()
                state[h] = (ps3.tile([P, 512], F32, tag="att", name="att"),
                            p_work.tile([P, 512], F16, tag="colsum",
                                        name="colsum"))
                for kt in range(nk):
                    # shorter pending queue only near the END of the very
                    # last head: early kts keep the deep pipeline (PE ahead
                    # of exp), the tail still drains early so the final
                    # output projection starts with less latency
                    lag = 4 if (qt == NQT - 1 and h == 3
                                and kt >= nk - 6) else LAG
                    # previous q-tile's output projection emitted mid-head;
                    # on long tiles spread it at four points so the filler
                    # matches the exp-bound score/attend cadence
                    if qt > 0:
                        pts = {8: (2, 4, 6, 7), 12: (3, 6, 9, 11),
                               16: (4, 8, 11, 14)}[nk]
                        if kt in pts:
                            o_proj_chunk(qt - 1, h, dts=(pts.index(kt),))
                    j = kt - 4 * qt
                    x0 = j * P if j >= 0 else 0
                    F = 512 - x0
                    sp = ps_s.tile([P, 512], F32, tag="s")
                    nc.tensor.matmul(
                        sp[:, x0:512],
                        lhsT=qkvb[:, 4, ds(kt * P, P)],
                        rhs=qkvb[:, h, ds(qt * 512 + x0, F)],
                        start=True, stop=True,
                    )
                    if kt % 4 == 0:
                        pr_quad = p_probs.tile([P, 4, 512], BF16, tag="probs")
                    pr = pr_quad[:, kt % 4, :]
                    nc.scalar.activation(
                        pr[:, x0:512], sp[:, x0:512],
                        mybir.ActivationFunctionType.Exp,
                        scale=EXP_SCALE, bias=ebias)
                    if j >= 0:
                        # zero the k>q half of the diagonal tile in place
                        # (local col c vs partition p: keep iff c >= p)
                        nc.gpsimd.affine_select(
                            out=pr[:, x0:512], in_=pr[:, x0:512],
                            pattern=[[1, F]], base=0,
                            channel_multiplier=-1,
                            compare_op=mybir.AluOpType.is_ge, fill=0.0)
                    pend.append((h, pr, x0, kt))
                    thr = 1 if qt == NQT - 1 else 3
                    while len(pend) > lag:
                        consume()
                        for e in fin_q:
                            e[1] += 1
                        if fin_q and fin_q[0][1] >= thr:
                            finalize(fin_q.popleft()[0])
            while pend:
                consume()
            if drain_filler is not None:
                # PE work emitted ahead of the remaining finalizes, whose
                # ones-matmuls stall in-order on the DVE colsum chain
                drain_filler()
            while fin_q:
                finalize(fin_q.popleft()[0])

        # ------- Fused phases: per token tile: projection+conv, then the
        # attention q-tile that just became computable, then the (pipelined)
        # output projection of the previous q-tile.
        def ht_alloc():
            return p_ht.tile([P, NPR, 2, 2, 512], FP8, tag="ht", name="ht")

        def ht_dispatch(ht, tt, chunks):
            for c0, w_ in chunks:
                nc.sync.dma_start(ht[:, ds(c0, w_)],
                                  h_d[:, tt, ds(c0, w_)])

        def conv_fc(tt, fc, pp, dve_copy=False):
            t0 = tt * 512
            # pre-conv x (64x) -> bf16 for the DVE conv taps. When run
            # as attention filler, the copy goes on DVE because ACT is
            # clogged with exp tiles there.
            if dve_copy:
                nc.vector.tensor_copy(qkvf[:, fc, ds(3 + t0, 512)], pp)
            else:
                nc.scalar.copy(qkvf[:, fc, ds(3 + t0, 512)], pp)
            # conv taps: out[t] = x[t] + sum_k x[t+k-3]*w[k].
            # Products via tensor_scalar (4x DVE mode — the tensor-tensor
            # variant gets no fast mode), sums via bf16 tensor_tensor (2x)
            ca = p_work.tile([P, 512], BF16, tag="ctmpa", name="ca")
            cb = p_work.tile([P, 512], BF16, tag="ctmpb", name="cb")
            nc.vector.tensor_scalar(
                ca, qkvf[:, fc, ds(t0 + 0, 512)],
                cw[:, fc * 4 + 0: fc * 4 + 1], None, op0=MULT)
            nc.vector.tensor_scalar(
                cb, qkvf[:, fc, ds(t0 + 1, 512)],
                cw[:, fc * 4 + 1: fc * 4 + 2], None, op0=MULT)
            nc.vector.tensor_add(ca, ca, cb)
            nc.vector.tensor_scalar(
                cb, qkvf[:, fc, ds(t0 + 2, 512)],
                cw[:, fc * 4 + 2: fc * 4 + 3], None, op0=MULT)
            nc.vector.tensor_add(ca, ca, cb)
            # last tap's weight is pre-biased +1 on the host, folding the
            # residual x[t] into the same tensor_scalar product
            nc.vector.tensor_scalar(
                cb, qkvf[:, fc, ds(t0 + 3, 512)],
                cw[:, fc * 4 + 3: fc * 4 + 4], None, op0=MULT)
            nc.vector.tensor_add(qkvb[:, fc, ts(tt, 512)], ca, cb)

        def proj_fc(tt, fc, ht, dve_copy=False):
            pp = ps2.tile([P, 512], F32, tag="proj", name="pp")
            # pr-major: each hidden pr chunk is fully consumed (all three
            # hi/lo terms) as soon as it lands, minimizing startup stalls
            k = 0
            for pr_ in range(NPR):
                for whl, hhl in ((0, 0), (0, 1), (1, 0)):
                    nc.tensor.matmul(
                        pp, lhsT=wq[:, fc, pr_, whl], rhs=ht[:, pr_, hhl],
                        start=(k == 0), stop=(k == 3 * NPR - 1),
                        perf_mode=DR)
                    k += 1
            conv_fc(tt, fc, pp, dve_copy)

        def transp_v(tt):
            # v (fc=5) of this token tile -> natural [token, dh] layout
            trp = ps1.tile([P, 512], BF16, tag="small")
            for j in range(4):
                nc.tensor.transpose(trp[:, ds(j * P, P)],
                                    qkvb[:, 5, ds((tt * 4 + j) * P, P)],
                                    ident)
            nc.vector.tensor_copy(vnat[:, ds(tt * 4, 4), :], trp)

        prefetched = None
        for tt in range(NTT):
            if tt == 0:
                # The DMA engine pool executes one transfer at a time, so
                # the dispatch sequence below is a global priority schedule:
                # interleave hidden pr-chunks with the weight packs in
                # exactly the order the paired pr-major projection consumes
                # them (fc4+fc5 first, pr chunk by pr chunk).
                ht = ht_alloc()
                ht_dispatch(ht, 0, ((0, 1),))
                nc.sync.dma_start(wq[:, 4, ds(0, 4)], w_d[:, 4, ds(0, 4)])
                nc.sync.dma_start(wq[:, 5, ds(0, 4)], w_d[:, 5, ds(0, 4)])
                ht_dispatch(ht, 0, ((1, 1), (2, 2)))
                nc.sync.dma_start(wq[:, 4, ds(4, 4)], w_d[:, 4, ds(4, 4)])
                nc.sync.dma_start(wq[:, 5, ds(4, 4)], w_d[:, 5, ds(4, 4)])
                ht_dispatch(ht, 0, ((4, 2), (6, 2)))
                nc.sync.dma_start(cw0, cw_d)
                nc.vector.tensor_copy(cw, cw0)
                for fc in (0, 1, 2, 3):
                    nc.sync.dma_start(wq[:, fc], w_d[:, fc])
                prefetched = ht_alloc()
                ht_dispatch(prefetched, 1, ((0, 4), (4, 4)))
            else:
                ht = prefetched
                if tt == 1:
                    nc.sync.dma_start(wo, wo_d)
                if tt < NTT - 1:
                    prefetched = ht_alloc()
                    ht_dispatch(prefetched, tt + 1, ((0, 4), (4, 4)))

            if tt == 0:
                # fc4+fc5 chains interleaved pr-major: two PSUM chains
                # consume each hidden pr chunk at the DMA supply rate; the
                # remaining projections ride inside the attention blocks
                pps = {4: ps2.tile([P, 512], F32, tag="proj", name="pp"),
                       5: ps2.tile([P, 512], F32, tag="proj", name="pp")}
                ks = {4: 0, 5: 0}
                for pr_ in range(NPR):
                    for fc in (4, 5):
                        for whl, hhl in ((0, 0), (0, 1), (1, 0)):
                            nc.tensor.matmul(
                                pps[fc], lhsT=wq[:, fc, pr_, whl],
                                rhs=ht[:, pr_, hhl],
                                start=(ks[fc] == 0),
                                stop=(ks[fc] == 3 * NPR - 1), perf_mode=DR)
                            ks[fc] += 1
                    # pad known DMA-supply stalls with warm matmuls so the
                    # PE p-state ramp survives the bandwidth-gated stretch
                    for _ in range({1: 2, 3: 4}.get(pr_, 0)):
                        wpad = ps_s.tile([P, 512], F32, tag="s", name="wpad")
                        nc.tensor.matmul(wpad, lhsT=wtiny, rhs=wscr,
                                         start=True, stop=True)
                conv_fc(0, 4, pps[4])
                conv_fc(0, 5, pps[5])
                proj_fc(0, 0, ht)
                proj_fc(0, 1, ht)
            else:
                for fc in (0, 1, 2, 3):
                    proj_fc(tt, fc, ht)
            transp_v(tt)
            # the next tile's k/v projections (fc4, fc5) ride as PE filler
            # inside this tile's attention: they only need the prefetched
            # hidden tile, and attention's exp-bound stretches absorb them
            fillers = {}
            if tt == 0:
                fillers[0] = lambda: proj_fc(0, 2, ht, dve_copy=True)
                fillers[1] = lambda: proj_fc(0, 3, ht, dve_copy=True)
            if tt < NTT - 1:
                nxt, pf = tt + 1, prefetched
                fillers[1 if tt else 2] = \
                    lambda n=nxt, p=pf: proj_fc(n, 4, p, dve_copy=True)
                fillers[3] = lambda n=nxt, p=pf: proj_fc(n, 5, p,
                                                         dve_copy=True)
            final_pr0 = {}
            if tt == NTT - 1:
                def drain_filler():
                    final_pr0[0] = o_proj_final_pr0(0)
                attn_B(tt, fillers=fillers, drain_filler=drain_filler)
            else:
                attn_B(tt, fillers=fillers)
        for t4 in range(4):
            o_proj_chunk(NQT - 1, t4, final=True,
                         pre_ops=final_pr0.get(t4))

    if legalize:
        _legalize_waits(nc)
    _CACHE[key] = nc
    return nc


def _prep_inputs(hidden_states, w_q, w_k, w_v, w_o, conv_w):
    """Build the 8 per-core input maps (host-side shard + fp8 hi/lo split)."""
    f8 = ml_dtypes.float8_e4m3

    def hpairs(x):  # [2048 d, 2048 t] -> [128, 4, 8, 2, 512]
        return np.ascontiguousarray(
            x.reshape(NPR, 2, P, NTT, 512).transpose(2, 3, 0, 1, 4))

    def wpairs(x):  # [2048, 768] -> [128, 6, 8, 2, 128]
        return np.ascontiguousarray(
            x.reshape(NPR, 2, P, NF, P).transpose(2, 3, 0, 1, 4))

    def split8(x):
        hi = x.astype(f8)
        lo = (x - hi.astype(np.float32)).astype(f8)
        return hi, lo

    # hidden split is shared by the 4 cores of a batch
    h_packs = []
    for b in range(2):
        hT = np.ascontiguousarray(hidden_states[b].T)
        hi, lo = split8(hT)
        h_packs.append(np.ascontiguousarray(
            np.stack([hpairs(hi), hpairs(lo)], axis=3)))

    in_maps = []
    for c in range(8):
        b, g = c // 4, c % 4
        wqkv = np.concatenate(
            [w_q[:, g * 512:(g + 1) * 512],
             w_k[:, g * 128:(g + 1) * 128],
             w_v[:, g * 128:(g + 1) * 128]], axis=1) * WSCALE
        w_hi, w_lo = split8(wqkv)
        w_pack = np.ascontiguousarray(
            np.stack([wpairs(w_hi), wpairs(w_lo)], axis=3))
        wop = np.ascontiguousarray(w_o[g * 512:(g + 1) * 512, :]) * WSCALE
        wo_hi, wo_lo = split8(wop)
        wo_pack = np.ascontiguousarray(np.stack(
            [wo_hi.reshape(2, 2, P, D).transpose(2, 0, 1, 3),
             wo_lo.reshape(2, 2, P, D).transpose(2, 0, 1, 3)], axis=1))
        cwc = np.concatenate(
            [conv_w[g * 512:(g + 1) * 512],
             conv_w[2048 + g * 128: 2048 + (g + 1) * 128],
             conv_w[2560 + g * 128: 2560 + (g + 1) * 128]], axis=0)  # [768,4]
        cwp = np.ascontiguousarray(
            cwc.reshape(NF, P, 4).transpose(1, 0, 2).reshape(P, NF * 4)
        ).astype(np.float32)
        # residual fold: out = x + sum_k x_k w_k == sum taps with w3 += 1
        cwp[:, 3::4] += 1.0
        in_maps.append({
            "h": h_packs[b],
            "w": w_pack,
            "wo": wo_pack,
            "conv_w": cwp,
        })
    return in_maps


def kernel(hidden_states, w_q, w_k, w_v, w_o, conv_w, _trace=False):
    nc = _build()
    in_maps = _prep_inputs(
        np.asarray(hidden_states, dtype=np.float32),
        np.asarray(w_q, dtype=np.float32),
        np.asarray(w_k, dtype=np.float32),
        np.asarray(w_v, dtype=np.float32),
        np.asarray(w_o, dtype=np.float32),
        np.asarray(conv_w, dtype=np.float32),
    )
    res = run_bass_kernel_spmd(nc, in_maps, core_ids=list(range(8)),
                               trace=_trace)
    outs = [r["out"] for r in res.results]
    full = np.empty((2, S, D), dtype=np.float32)
    for b in range(2):
        acc = (outs[4 * b].astype(np.float32)
               + outs[4 * b + 1].astype(np.float32)
               + outs[4 * b + 2].astype(np.float32)
               + outs[4 * b + 3].astype(np.float32))
        full[b] = acc * (1.0 / OUT_DIV)
    if _trace:
        kernel.last_results = res
    return full


# revision 100
# speedup vs baseline: 1.0014x; 1.0005x over previous
"""CanonCausalMultiheadAttn Trainium2 kernel (fp8 DoubleRow version).

Sharding: 8 cores = 2 (batch) x 4 (kv-head groups). Core c handles batch
c//4 and kv-group g=c%4 (q heads 4g..4g+3, kv head g). w_q/w_k/w_v are
column-sharded by head group, w_o row-sharded; each core emits a partial
[S, D] output (bf16) which the host sums over the 4 groups of its batch.

The four heads of each q-tile are software-pipelined into one flat
(head, k-tile) sequence with a deep (LAG=14) pending-probs queue, so the
in-order PE queue never stalls on the exp (ACT) -> causal-mask (Pool)
producer chain; the previous q-tile's output projection is interleaved as
PE filler. All DMAs serialize through one shared HWDGE (~625ns dispatch
each), so hi/lo fp8 planes are PACKED into single DRAM tensors and output
tiles ship as one wide DMA per 128-token chunk. A short chain of warm-up
matmuls primes the PE p-state ramp (half clock until 3us continuously
busy) while the first input DMAs are in flight.

Per-core dataflow (transposed [feature, token] layout; v transposed on PE):
  qkvT[f, t] = w_qkv[:, f].T @ hT[:, t]   -- fp8e4m3 DoubleRow matmuls with
      3-term hi/lo compensation (w_hi.h_hi + w_hi.h_lo + w_lo.h_hi), pr-major
      so each hidden chunk is fully consumed as it lands.
  conv: depthwise causal taps in bf16 on DVE (tensor_scalar products,
      tensor_tensor sums); conv weights stay f32.
  scores.T[k, q] = kT.T @ qT (bf16) -> exp on ACT (scale folds the fp8
      pre-scales; bias -2ln2 keeps fp16 column sums in range)
  causal: k-tiles with k0 <= q_end only; diagonal tiles masked in-place
      by an affine_select on the (otherwise idle) Pool engine.
  attT[dh, q] += v_nat[k,:].T @ probsT  (bf16)
  colsum via DVE adds in fp16 (2x DVE mode), partition-reduced by one
      fp16 ones-matmul (ones=4.0 folds the attT scale correction).
  attT stored as fp8 hi+lo; out[t, d] = attT.T @ w_o_rows via 3-term DR,
      shipped bf16 (divided by 1024x net scale on the host).

Scales: w_qkv and w_o are pre-scaled x64 into fp8 (e4m3 = IEEE variant,
  max finite 240); hidden stays x1. qkv = 64x, scores = 4096 s (folded
  into exp scale), probs = p/4 (exp bias), att = 16*Sum p v, colsum = p/4
  summed, ones=4.0 => attT = 16*attended, out = 1024*true. Host divides.
"""

import numpy as np
import ml_dtypes
from collections import deque
from contextlib import ExitStack

import concourse.bass as bass
import concourse.tile as tile
import concourse.mybir as mybir
from concourse.bass import ds, ts
from concourse.bass_utils import run_bass_kernel_spmd
from concourse.masks import make_identity

BF16 = mybir.dt.bfloat16
F16 = mybir.dt.float16
F32 = mybir.dt.float32
FP8 = mybir.dt.float8e4
DR = mybir.MatmulPerfMode.DoubleRow
P = 128
S = 2048          # sequence length
D = 2048          # d_model
NF = 6            # feature chunks of 128: 4 q heads, 1 k, 1 v
NPR = 8           # DR contraction pairs over d_model (2048 = 8*256)
NQT = S // 512    # 4 query tiles of 512
NTT = S // 512    # 4 token tiles of 512
WSCALE = 64.0     # fp8 pre-scale on w_qkv and w_o (e4m3 max is 240)
ISQ = 1.0 / np.sqrt(128.0)
EXP_SCALE = ISQ / (WSCALE * WSCALE)   # scores PSUM holds 4096*s
EXP_BIAS = float(-2.0 * np.log(2.0))  # probs = p/4 (fp16 colsum headroom)
OUT_DIV = 1024.0  # 16 (attT) * 64 (w_o)
WARM = 7          # PE p-state warm-up matmuls
MULT = mybir.AluOpType.mult
ADD = mybir.AluOpType.add

_CACHE = {}


def _legalize_waits(nc):
    """Split multi-wait sync_info into preceding single-wait engine NOPs.

    The walrus codegen in this container accepts at most ONE sync wait per
    TPB instruction ("Too many sync wait commands"), but the Tile scheduler
    freely emits several. An engine executes its queue in order, so hoisting
    the extra waits onto NoOps right before the instruction is equivalent.
    """
    n = 0
    for f in nc.m.functions:
        for blk in f.blocks:
            out = []
            changed = False
            for inst in blk.instructions:
                si = inst.sync_info
                if (si is not None and si.on_wait and len(si.on_wait) > 1
                        and str(inst.engine) != "EngineType.Unassigned"):
                    waits = list(si.on_wait)
                    for w in waits[:-1]:
                        out.append(mybir.InstNoOp(
                            name=f"I-wf{n}", engine=inst.engine, ins=[],
                            outs=[],
                            sync_info=mybir.SyncInfo(on_wait=[w],
                                                     on_update=[])))
                        n += 1
                    si.on_wait = [waits[-1]]
                    changed = True
                out.append(inst)
            if changed:
                blk.instructions = out
    return n


def _build(legalize=True):
    key = "nc" if legalize else "nc_raw"
    if key in _CACHE:
        return _CACHE[key]
    nc = bass.Bass("TRN2", target_bir_lowering=False, debug=False)

    # hi/lo fp8 planes packed into single DRAM tensors: every DMA dispatch
    # serializes through one shared HWDGE (~625ns), so fewer+wider wins
    h_d = nc.dram_tensor("h", [P, NTT, NPR, 2, 2, 512], FP8,
                         kind="ExternalInput").ap()
    w_d = nc.dram_tensor("w", [P, NF, NPR, 2, 2, P], FP8,
                         kind="ExternalInput").ap()
    wo_d = nc.dram_tensor("wo", [P, 2, 2, 2, D], FP8,
                          kind="ExternalInput").ap()
    cw_d = nc.dram_tensor("conv_w", [P, NF * 4], F32,
                          kind="ExternalInput").ap()
    out_d = nc.dram_tensor("out", [S, D], BF16, kind="ExternalOutput").ap()

    out_v = out_d.rearrange("(po pi) d -> pi po d", pi=P)      # [128,16,2048]

    with tile.TileContext(nc) as tc, ExitStack() as ctx:
        const = ctx.enter_context(tc.tile_pool(name="const", bufs=1))
        p_ht = ctx.enter_context(tc.tile_pool(name="ht", bufs=2))
        p_work = ctx.enter_context(tc.tile_pool(name="work", bufs=3))
        p_probs = ctx.enter_context(tc.tile_pool(name="probs", bufs=6))
        p_out = ctx.enter_context(tc.tile_pool(name="outp", bufs=4))
        ps2 = ctx.enter_context(tc.tile_pool(name="ps2", bufs=2, space="PSUM"))
        ps_s = ctx.enter_context(tc.tile_pool(name="ps_s", bufs=3,
                                              space="PSUM"))
        ps3 = ctx.enter_context(tc.tile_pool(name="ps3", bufs=2, space="PSUM"))
        ps1 = ctx.enter_context(tc.tile_pool(name="ps1", bufs=1, space="PSUM"))

        # --- constants / persistent tensors ---
        # tiny warm operand memset FIRST on Pool (~100ns) so the PE p-state
        # warm-up starts ~1us before make_identity would allow
        wtiny = const.tile([P, P], BF16, tag="wtiny")
        nc.gpsimd.memset(wtiny, 0.5)
        ident = const.tile([P, P], BF16, tag="ident")
        make_identity(nc, ident)
        wscr = const.tile([P, 512], BF16, tag="wscr")
        nc.vector.memset(wscr, 0.5)
        # PE p-state warm-up: the tensor engine runs at half clock until it
        # has been continuously busy 3us; burn that ramp on dummies while
        # the first input DMAs are still in flight.
        wps = ps_s.tile([P, 512], F32, tag="s", name="warm")
        for _ in range(8):
            nc.tensor.matmul(wps[:, 0:P], lhsT=wtiny, rhs=wtiny,
                             start=True, stop=True)
        for _ in range(WARM):
            nc.tensor.matmul(wps, lhsT=wtiny, rhs=wscr, start=True, stop=True)
        cw0 = const.tile([P, NF * 4], F32, tag="cw0")
        cw = const.tile([P, NF * 4], F32, tag="cw")
        wq = const.tile([P, NF, NPR, 2, 2, P], FP8, tag="wq")
        wo = const.tile([P, 2, 2, 2, D], FP8, tag="wo")
        # raw (pre-conv) qkv.T in bf16 (64x scale), 3 leading zero columns so
        # the causal conv taps can read t-3..t-1 without edge cases
        qkvf = const.tile([P, NF, S + 3], BF16, tag="qkvf")
        nc.gpsimd.memset(qkvf[:, :, 0:3], 0.0)
        qkvb = const.tile([P, NF, S], BF16, tag="qkvb")    # conv'd qkv.T
        vnat = const.tile([P, 16, P], BF16, tag="vnat")    # v in [token, dh]
        atth = const.tile([P, 4, S], FP8, tag="atth")      # attT hi per head
        attl = const.tile([P, 4, S], FP8, tag="attl")      # attT lo per head
        ones2 = const.tile([P, P], F16, tag="ones2")
        nc.vector.memset(ones2, 4.0)
        ebias = const.tile([P, 1], F32, tag="ebias")
        nc.vector.memset(ebias, EXP_BIAS)

        obuf = {}  # t4 -> wide bf16 output tile (one DMA per token chunk)

        def oproj_trio(tt16, op, dt, pr_, k0):
            # atth terms first: the attl piece lands one DVE op later in
            # the finalize chain, so leading with atth starts ~190ns sooner
            for k, (lhs, hl) in enumerate(
                    ((atth, 0), (atth, 1), (attl, 0))):
                nc.tensor.matmul(
                    op,
                    lhsT=lhs[:, ds(2 * pr_, 2), ds(tt16 * P, P)],
                    rhs=wo[:, hl, pr_, :, ds(dt * 512, 512)],
                    start=(k0 + k == 0), stop=(k0 + k == 5),
                    perf_mode=DR)

        def o_proj_final_pr0(t4):
            # head-0/1 trios of a final chunk: eligible as soon as the
            # first head pair is normalized, used as PE filler while the
            # last head's denominator drains on DVE
            tt16 = (NQT - 1) * 4 + t4
            ops = {}
            for dt in range(4):
                if dt % 2 == 1:
                    ops[dt] = ps_s.tile([P, 512], F32, tag="s", name="op")
                else:
                    ops[dt] = ps2.tile([P, 512], F32, tag="proj", name="op")
                oproj_trio(tt16, ops[dt], dt, 0, 0)
            return ops

        def o_proj_chunk(qt, t4, final=False, dts=(0, 1, 2, 3),
                         pre_ops=None):
            # output projection for one token-128-tile of q-tile qt; the
            # PSUM->SBUF copies land in one wide bf16 tile which ships as a
            # SINGLE DMA per token chunk (HWDGE dispatch is the scarce
            # resource, not DMA bandwidth)
            tt16 = qt * 4 + t4
            if t4 not in obuf:
                obuf[t4] = p_out.tile([P, 4, 512], BF16, tag="ob", name="ob")
            ob = obuf[t4]

            def trio(op, dt, pr_, k0):
                oproj_trio(tt16, op, dt, pr_, k0)

            for dt in dts:
                if pre_ops is not None:
                    op = pre_ops[dt]
                else:
                    if final and dt % 2 == 1:
                        op = ps_s.tile([P, 512], F32, tag="s", name="op")
                    else:
                        op = ps2.tile([P, 512], F32, tag="proj", name="op")
                    trio(op, dt, 0, 0)
                trio(op, dt, 1, 3)
                if final:
                    # alternate engines so the last chunk's copies overlap;
                    # ship in pieces so earlier DMAs overlap later matmuls
                    if dt == 3 and t4 == 3:
                        # the very last copy splits across ACT+DVE halves
                        # running in parallel (~390ns instead of 612)
                        nc.scalar.copy(ob[:, dt, ds(0, 256)],
                                       op[:, ds(0, 256)])
                        nc.vector.tensor_copy(ob[:, dt, ds(256, 256)],
                                              op[:, ds(256, 256)])
                    elif dt % 2 == 0:
                        nc.vector.tensor_copy(ob[:, dt, :], op)
                    else:
                        nc.scalar.copy(ob[:, dt, :], op)
                    if dt == 1:
                        nc.sync.dma_start(out_v[:, tt16, ds(0, 1024)],
                                          ob[:, ds(0, 2)])
                    elif dt == 2:
                        nc.sync.dma_start(out_v[:, tt16, ds(1024, 512)],
                                          ob[:, ds(2, 1)])
                    elif dt == 3:
                        nc.sync.dma_start(out_v[:, tt16, ds(1536, 512)],
                                          ob[:, ds(3, 1)])
                elif dt == 3:
                    nc.scalar.copy(ob[:, dt, :], op)
                else:
                    nc.vector.tensor_copy(ob[:, dt, :], op)
            if dts[-1] == 3:
                if not final:
                    nc.sync.dma_start(out_v[:, tt16, :], ob)
                del obuf[t4]

        def attn_B(qt, fillers=None, drain_filler=None):
            # attention for q-tile qt (needs token tiles <= qt). The four
            # heads are software-pipelined into ONE flat (h, kt) sequence:
            # scores of head h+1 are emitted while head h's attended matmuls
            # drain, so the in-order PE queue never stalls on the
            # exp (ACT) -> mask (Pool) producer chain. The previous q-tile's
            # output projection is interleaved as additional PE filler.
            nk = 4 * (qt + 1)
            LAG = 14
            state = {}  # h -> (att, colsum)
            fin = {}    # h -> (att, colsum) awaiting denominator finalize
            fin_q = deque()  # [h, consumes-since-ready]
            pend = deque()
            pr_quad = None

            def consume():
                ch, ppr, px0, pkt = pend.popleft()
                att, colsum = state[ch] if ch in state else fin[ch]
                nc.tensor.matmul(
                    att[:, px0:512], lhsT=vnat[:, pkt, :],
                    rhs=ppr[:, px0:512],
                    start=(pkt == 0), stop=(pkt == nk - 1))
                # softmax denominator: accumulate exp'd probs on DVE
                # (partition dim reduced by ONE ones-matmul at the end)
                if pkt == 0:
                    nc.vector.tensor_copy(colsum, ppr)
                else:
                    nc.vector.tensor_add(
                        colsum[:, px0:512], colsum[:, px0:512],
                        ppr[:, px0:512])
                if pkt == nk - 1:
                    fin[ch] = state.pop(ch)
                    fin_q.append([ch, 0])

            def finalize(ch):
                att, colsum = fin[ch]
                smp = ps1.tile([P, 512], F32, tag="small")
                rec = p_work.tile([P, 512], F32, tag="rec")
                t16 = p_work.tile([P, 512], F16, tag="t16")
                if qt == NQT - 1 and ch >= 2:
                    # last heads before the final output projection: the
                    # whole normalization chain runs in 128-col pieces
                    # (ones-matmul and reciprocal included) so the first
                    # final o_proj trios start ~0.6us sooner
                    for pc in range(4):
                        c = ds(pc * P, P)
                        nc.tensor.matmul(smp[:, c], lhsT=ones2,
                                         rhs=colsum[:, c],
                                         start=True, stop=True)
                        nc.vector.reciprocal(rec[:, c], smp[:, c])
                        nc.vector.tensor_mul(t16[:, c], att[:, c], rec[:, c])
                        nc.vector.tensor_copy(
                            atth[:, ch, ds(qt * 512 + pc * P, P)], t16[:, c])
                        nc.vector.tensor_sub(
                            attl[:, ch, ds(qt * 512 + pc * P, P)], t16[:, c],
                            atth[:, ch, ds(qt * 512 + pc * P, P)])
                else:
                    nc.tensor.matmul(smp, lhsT=ones2, rhs=colsum,
                                     start=True, stop=True)
                    nc.vector.reciprocal(rec, smp)
                    nc.vector.tensor_mul(t16, att, rec)
                    nc.gpsimd.tensor_copy(atth[:, ch, ts(qt, 512)], t16)
                    nc.gpsimd.tensor_sub(attl[:, ch, ts(qt, 512)], t16,
                                         atth[:, ch, ts(qt, 512)])
                del fin[ch]

            for h in range(4):
                if fillers and h in fillers:
                    fillers[h]()
                state[h] = (ps3.tile([P, 512], F32, tag="att", name="att"),
                            p_work.tile([P, 512], F16, tag="colsum",
                                        name="colsum"))
                for kt in range(nk):
                    # shorter pending queue only near the END of the very
                    # last head: early kts keep the deep pipeline (PE ahead
                    # of exp), the tail still drains early so the final
                    # output projection starts with less latency
                    lag = 4 if (qt == NQT - 1 and h == 3
                                and kt >= nk - 6) else LAG
                    # previous q-tile's output projection emitted mid-head;
                    # on long tiles spread it at four points so the filler
                    # matches the exp-bound score/attend cadence
                    if qt > 0:
                        pts = {8: (2, 4, 6, 7), 12: (3, 6, 9, 11),
                               16: (4, 8, 12, 15)}[nk]
                        if kt in pts:
                            o_proj_chunk(qt - 1, h, dts=(pts.index(kt),))
                    j = kt - 4 * qt
                    x0 = j * P if j >= 0 else 0
                    F = 512 - x0
                    sp = ps_s.tile([P, 512], F32, tag="s")
                    nc.tensor.matmul(
                        sp[:, x0:512],
                        lhsT=qkvb[:, 4, ds(kt * P, P)],
                        rhs=qkvb[:, h, ds(qt * 512 + x0, F)],
                        start=True, stop=True,
                    )
                    if kt % 4 == 0:
                        pr_quad = p_probs.tile([P, 4, 512], BF16, tag="probs")
                    pr = pr_quad[:, kt % 4, :]
                    nc.scalar.activation(
                        pr[:, x0:512], sp[:, x0:512],
                        mybir.ActivationFunctionType.Exp,
                        scale=EXP_SCALE, bias=ebias)
                    if j >= 0:
                        # zero the k>q half of the diagonal tile in place
                        # (local col c vs partition p: keep iff c >= p)
                        nc.gpsimd.affine_select(
                            out=pr[:, x0:512], in_=pr[:, x0:512],
                            pattern=[[1, F]], base=0,
                            channel_multiplier=-1,
                            compare_op=mybir.AluOpType.is_ge, fill=0.0)
                    pend.append((h, pr, x0, kt))
                    thr = 1 if qt == NQT - 1 else 3
                    while len(pend) > lag:
                        consume()
                        for e in fin_q:
                            e[1] += 1
                        if fin_q and fin_q[0][1] >= thr:
                            finalize(fin_q.popleft()[0])
            while pend:
                consume()
            if drain_filler is not None:
                # PE work emitted ahead of the remaining finalizes, whose
                # ones-matmuls stall in-order on the DVE colsum chain
                drain_filler()
            while fin_q:
                finalize(fin_q.popleft()[0])

        # ------- Fused phases: per token tile: projection+conv, then the
        # attention q-tile that just became computable, then the (pipelined)
        # output projection of the previous q-tile.
        def ht_alloc():
            return p_ht.tile([P, NPR, 2, 2, 512], FP8, tag="ht", name="ht")

        def ht_dispatch(ht, tt, chunks):
            for c0, w_ in chunks:
                nc.sync.dma_start(ht[:, ds(c0, w_)],
                                  h_d[:, tt, ds(c0, w_)])

        def conv_fc(tt, fc, pp, dve_copy=False):
            t0 = tt * 512
            # pre-conv x (64x) -> bf16 for the DVE conv taps. When run
            # as attention filler, the copy goes on DVE because ACT is
            # clogged with exp tiles there.
            if dve_copy:
                nc.vector.tensor_copy(qkvf[:, fc, ds(3 + t0, 512)], pp)
            else:
                nc.scalar.copy(qkvf[:, fc, ds(3 + t0, 512)], pp)
            # conv taps: out[t] = x[t] + sum_k x[t+k-3]*w[k].
            # Products via tensor_scalar (4x DVE mode — the tensor-tensor
            # variant gets no fast mode), sums via bf16 tensor_tensor (2x)
            ca = p_work.tile([P, 512], BF16, tag="ctmpa", name="ca")
            cb = p_work.tile([P, 512], BF16, tag="ctmpb", name="cb")
            nc.vector.tensor_scalar(
                ca, qkvf[:, fc, ds(t0 + 0, 512)],
                cw[:, fc * 4 + 0: fc * 4 + 1], None, op0=MULT)
            nc.vector.tensor_scalar(
                cb, qkvf[:, fc, ds(t0 + 1, 512)],
                cw[:, fc * 4 + 1: fc * 4 + 2], None, op0=MULT)
            nc.vector.tensor_add(ca, ca, cb)
            nc.vector.tensor_scalar(
                cb, qkvf[:, fc, ds(t0 + 2, 512)],
                cw[:, fc * 4 + 2: fc * 4 + 3], None, op0=MULT)
            nc.vector.tensor_add(ca, ca, cb)
            # last tap's weight is pre-biased +1 on the host, folding the
            # residual x[t] into the same tensor_scalar product
            nc.vector.tensor_scalar(
                cb, qkvf[:, fc, ds(t0 + 3, 512)],
                cw[:, fc * 4 + 3: fc * 4 + 4], None, op0=MULT)
            nc.vector.tensor_add(qkvb[:, fc, ts(tt, 512)], ca, cb)

        def proj_fc(tt, fc, ht, dve_copy=False):
            pp = ps2.tile([P, 512], F32, tag="proj", name="pp")
            # pr-major: each hidden pr chunk is fully consumed (all three
            # hi/lo terms) as soon as it lands, minimizing startup stalls
            k = 0
            for pr_ in range(NPR):
                for whl, hhl in ((0, 0), (0, 1), (1, 0)):
                    nc.tensor.matmul(
                        pp, lhsT=wq[:, fc, pr_, whl], rhs=ht[:, pr_, hhl],
                        start=(k == 0), stop=(k == 3 * NPR - 1),
                        perf_mode=DR)
                    k += 1
            conv_fc(tt, fc, pp, dve_copy)

        def transp_v(tt):
            # v (fc=5) of this token tile -> natural [token, dh] layout
            trp = ps1.tile([P, 512], BF16, tag="small")
            for j in range(4):
                nc.tensor.transpose(trp[:, ds(j * P, P)],
                                    qkvb[:, 5, ds((tt * 4 + j) * P, P)],
                                    ident)
            nc.vector.tensor_copy(vnat[:, ds(tt * 4, 4), :], trp)

        prefetched = None
        for tt in range(NTT):
            if tt == 0:
                # The DMA engine pool executes one transfer at a time, so
                # the dispatch sequence below is a global priority schedule:
                # interleave hidden pr-chunks with the weight packs in
                # exactly the order the paired pr-major projection consumes
                # them (fc4+fc5 first, pr chunk by pr chunk).
                ht = ht_alloc()
                ht_dispatch(ht, 0, ((0, 1),))
                nc.sync.dma_start(wq[:, 4, ds(0, 4)], w_d[:, 4, ds(0, 4)])
                nc.sync.dma_start(wq[:, 5, ds(0, 4)], w_d[:, 5, ds(0, 4)])
                ht_dispatch(ht, 0, ((1, 1), (2, 2)))
                nc.sync.dma_start(wq[:, 4, ds(4, 4)], w_d[:, 4, ds(4, 4)])
                nc.sync.dma_start(wq[:, 5, ds(4, 4)], w_d[:, 5, ds(4, 4)])
                ht_dispatch(ht, 0, ((4, 2), (6, 2)))
                nc.sync.dma_start(cw0, cw_d)
                nc.vector.tensor_copy(cw, cw0)
                for fc in (0, 1, 2, 3):
                    nc.sync.dma_start(wq[:, fc], w_d[:, fc])
                prefetched = ht_alloc()
                ht_dispatch(prefetched, 1, ((0, 4), (4, 4)))
            else:
                ht = prefetched
                if tt == 1:
                    nc.sync.dma_start(wo, wo_d)
                if tt < NTT - 1:
                    prefetched = ht_alloc()
                    ht_dispatch(prefetched, tt + 1, ((0, 4), (4, 4)))

            if tt == 0:
                # fc4+fc5 chains interleaved pr-major: two PSUM chains
                # consume each hidden pr chunk at the DMA supply rate; the
                # remaining projections ride inside the attention blocks
                pps = {4: ps2.tile([P, 512], F32, tag="proj", name="pp"),
                       5: ps2.tile([P, 512], F32, tag="proj", name="pp")}
                ks = {4: 0, 5: 0}
                for pr_ in range(NPR):
                    for fc in (4, 5):
                        for whl, hhl in ((0, 0), (0, 1), (1, 0)):
                            nc.tensor.matmul(
                                pps[fc], lhsT=wq[:, fc, pr_, whl],
                                rhs=ht[:, pr_, hhl],
                                start=(ks[fc] == 0),
                                stop=(ks[fc] == 3 * NPR - 1), perf_mode=DR)
                            ks[fc] += 1
                    # pad known DMA-supply stalls with warm matmuls so the
                    # PE p-state ramp survives the bandwidth-gated stretch
                    for _ in range({1: 2, 3: 4}.get(pr_, 0)):
                        wpad = ps_s.tile([P, 512], F32, tag="s", name="wpad")
                        nc.tensor.matmul(wpad, lhsT=wtiny, rhs=wscr,
                                         start=True, stop=True)
                conv_fc(0, 4, pps[4])
                conv_fc(0, 5, pps[5])
                proj_fc(0, 0, ht)
                proj_fc(0, 1, ht)
            else:
                for fc in (0, 1, 2, 3):
                    proj_fc(tt, fc, ht)
            transp_v(tt)
            # the next tile's k/v projections (fc4, fc5) ride as PE filler
            # inside this tile's attention: they only need the prefetched
            # hidden tile, and attention's exp-bound stretches absorb them
            fillers = {}
            if tt == 0:
                fillers[0] = lambda: proj_fc(0, 2, ht, dve_copy=True)
                fillers[1] = lambda: proj_fc(0, 3, ht, dve_copy=True)
            if tt < NTT - 1:
                nxt, pf = tt + 1, prefetched
                fillers[1 if tt else 2] = \
                    lambda n=nxt, p=pf: proj_fc(n, 4, p, dve_copy=True)
                fillers[3] = lambda n=nxt, p=pf: proj_fc(n, 5, p,
                                                         dve_copy=True)
            final_pr0 = {}
            if tt == NTT - 1:
                def drain_filler():
                    final_pr0[0] = o_proj_final_pr0(0)
                attn_B(tt, fillers=fillers, drain_filler=drain_filler)
            else:
                attn_B(tt, fillers=fillers)
        for t4 in range(4):
            o_proj_chunk(NQT - 1, t4, final=True,
                         pre_ops=final_pr0.get(t4))

    if legalize:
        _legalize_waits(nc)
    _CACHE[key] = nc
    return nc


def _prep_inputs(hidden_states, w_q, w_k, w_v, w_o, conv_w):
    """Build the 8 per-core input maps (host-side shard + fp8 hi/lo split)."""
    f8 = ml_dtypes.float8_e4m3

    def hpairs(x):  # [2048 d, 2048 t] -> [128, 4, 8, 2, 512]
        return np.ascontiguousarray(
            x.reshape(NPR, 2, P, NTT, 512).transpose(2, 3, 0, 1, 4))

    def wpairs(x):  # [2048, 768] -> [128, 6, 8, 2, 128]
        return np.ascontiguousarray(
            x.reshape(NPR, 2, P, NF, P).transpose(2, 3, 0, 1, 4))

    def split8(x):
        hi = x.astype(f8)
        lo = (x - hi.astype(np.float32)).astype(f8)
        return hi, lo

    # hidden split is shared by the 4 cores of a batch
    h_packs = []
    for b in range(2):
        hT = np.ascontiguousarray(hidden_states[b].T)
        hi, lo = split8(hT)
        h_packs.append(np.ascontiguousarray(
            np.stack([hpairs(hi), hpairs(lo)], axis=3)))

    in_maps = []
    for c in range(8):
        b, g = c // 4, c % 4
        wqkv = np.concatenate(
            [w_q[:, g * 512:(g + 1) * 512],
             w_k[:, g * 128:(g + 1) * 128],
             w_v[:, g * 128:(g + 1) * 128]], axis=1) * WSCALE
        w_hi, w_lo = split8(wqkv)
        w_pack = np.ascontiguousarray(
            np.stack([wpairs(w_hi), wpairs(w_lo)], axis=3))
        wop = np.ascontiguousarray(w_o[g * 512:(g + 1) * 512, :]) * WSCALE
        wo_hi, wo_lo = split8(wop)
        wo_pack = np.ascontiguousarray(np.stack(
            [wo_hi.reshape(2, 2, P, D).transpose(2, 0, 1, 3),
             wo_lo.reshape(2, 2, P, D).transpose(2, 0, 1, 3)], axis=1))
        cwc = np.concatenate(
            [conv_w[g * 512:(g + 1) * 512],
             conv_w[2048 + g * 128: 2048 + (g + 1) * 128],
             conv_w[2560 + g * 128: 2560 + (g + 1) * 128]], axis=0)  # [768,4]
        cwp = np.ascontiguousarray(
            cwc.reshape(NF, P, 4).transpose(1, 0, 2).reshape(P, NF * 4)
        ).astype(np.float32)
        # residual fold: out = x + sum_k x_k w_k == sum taps with w3 += 1
        cwp[:, 3::4] += 1.0
        in_maps.append({
            "h": h_packs[b],
            "w": w_pack,
            "wo": wo_pack,
            "conv_w": cwp,
        })
    return in_maps


def kernel(hidden_states, w_q, w_k, w_v, w_o, conv_w, _trace=False):
    nc = _build()
    in_maps = _prep_inputs(
        np.asarray(hidden_states, dtype=np.float32),
        np.asarray(w_q, dtype=np.float32),
        np.asarray(w_k, dtype=np.float32),
        np.asarray(w_v, dtype=np.float32),
        np.asarray(w_o, dtype=np.float32),
        np.asarray(conv_w, dtype=np.float32),
    )
    res = run_bass_kernel_spmd(nc, in_maps, core_ids=list(range(8)),
                               trace=_trace)
    outs = [r["out"] for r in res.results]
    full = np.empty((2, S, D), dtype=np.float32)
    for b in range(2):
        acc = (outs[4 * b].astype(np.float32)
               + outs[4 * b + 1].astype(np.float32)
               + outs[4 * b + 2].astype(np.float32)
               + outs[4 * b + 3].astype(np.float32))
        full[b] = acc * (1.0 / OUT_DIV)
    if _trace:
        kernel.last_results = res
    return full


# revision 102
# speedup vs baseline: 1.0017x; 1.0003x over previous
"""CanonCausalMultiheadAttn Trainium2 kernel (fp8 DoubleRow version).

Sharding: 8 cores = 2 (batch) x 4 (kv-head groups). Core c handles batch
c//4 and kv-group g=c%4 (q heads 4g..4g+3, kv head g). w_q/w_k/w_v are
column-sharded by head group, w_o row-sharded; each core emits a partial
[S, D] output (bf16) which the host sums over the 4 groups of its batch.

The four heads of each q-tile are software-pipelined into one flat
(head, k-tile) sequence with a deep (LAG=14) pending-probs queue, so the
in-order PE queue never stalls on the exp (ACT) -> causal-mask (Pool)
producer chain; the previous q-tile's output projection is interleaved as
PE filler. All DMAs serialize through one shared HWDGE (~625ns dispatch
each), so hi/lo fp8 planes are PACKED into single DRAM tensors and output
tiles ship as one wide DMA per 128-token chunk. A short chain of warm-up
matmuls primes the PE p-state ramp (half clock until 3us continuously
busy) while the first input DMAs are in flight.

Per-core dataflow (transposed [feature, token] layout; v transposed on PE):
  qkvT[f, t] = w_qkv[:, f].T @ hT[:, t]   -- fp8e4m3 DoubleRow matmuls with
      3-term hi/lo compensation (w_hi.h_hi + w_hi.h_lo + w_lo.h_hi), pr-major
      so each hidden chunk is fully consumed as it lands.
  conv: depthwise causal taps in bf16 on DVE (tensor_scalar products,
      tensor_tensor sums); conv weights stay f32.
  scores.T[k, q] = kT.T @ qT (bf16) -> exp on ACT (scale folds the fp8
      pre-scales; bias -2ln2 keeps fp16 column sums in range)
  causal: k-tiles with k0 <= q_end only; diagonal tiles masked in-place
      by an affine_select on the (otherwise idle) Pool engine.
  attT[dh, q] += v_nat[k,:].T @ probsT  (bf16)
  colsum via DVE adds in fp16 (2x DVE mode), partition-reduced by one
      fp16 ones-matmul (ones=4.0 folds the attT scale correction).
  attT stored as fp8 hi+lo; out[t, d] = attT.T @ w_o_rows via 3-term DR,
      shipped bf16 (divided by 1024x net scale on the host).

Scales: w_qkv and w_o are pre-scaled x64 into fp8 (e4m3 = IEEE variant,
  max finite 240); hidden stays x1. qkv = 64x, scores = 4096 s (folded
  into exp scale), probs = p/4 (exp bias), att = 16*Sum p v, colsum = p/4
  summed, ones=4.0 => attT = 16*attended, out = 1024*true. Host divides.
"""

import numpy as np
import ml_dtypes
from collections import deque
from contextlib import ExitStack

import concourse.bass as bass
import concourse.tile as tile
import concourse.mybir as mybir
from concourse.bass import ds, ts
from concourse.bass_utils import run_bass_kernel_spmd
from concourse.masks import make_identity

BF16 = mybir.dt.bfloat16
F16 = mybir.dt.float16
F32 = mybir.dt.float32
FP8 = mybir.dt.float8e4
DR = mybir.MatmulPerfMode.DoubleRow
P = 128
S = 2048          # sequence length
D = 2048          # d_model
NF = 6            # feature chunks of 128: 4 q heads, 1 k, 1 v
NPR = 8           # DR contraction pairs over d_model (2048 = 8*256)
NQT = S // 512    # 4 query tiles of 512
NTT = S // 512    # 4 token tiles of 512
WSCALE = 64.0     # fp8 pre-scale on w_qkv and w_o (e4m3 max is 240)
ISQ = 1.0 / np.sqrt(128.0)
EXP_SCALE = ISQ / (WSCALE * WSCALE)   # scores PSUM holds 4096*s
EXP_BIAS = float(-2.0 * np.log(2.0))  # probs = p/4 (fp16 colsum headroom)
OUT_DIV = 1024.0  # 16 (attT) * 64 (w_o)
WARM = 7          # PE p-state warm-up matmuls
MULT = mybir.AluOpType.mult
ADD = mybir.AluOpType.add

_CACHE = {}


def _legalize_waits(nc):
    """Split multi-wait sync_info into preceding single-wait engine NOPs.

    The walrus codegen in this container accepts at most ONE sync wait per
    TPB instruction ("Too many sync wait commands"), but the Tile scheduler
    freely emits several. An engine executes its queue in order, so hoisting
    the extra waits onto NoOps right before the instruction is equivalent.
    """
    n = 0
    for f in nc.m.functions:
        for blk in f.blocks:
            out = []
            changed = False
            for inst in blk.instructions:
                si = inst.sync_info
                if (si is not None and si.on_wait and len(si.on_wait) > 1
                        and str(inst.engine) != "EngineType.Unassigned"):
                    waits = list(si.on_wait)
                    for w in waits[:-1]:
                        out.append(mybir.InstNoOp(
                            name=f"I-wf{n}", engine=inst.engine, ins=[],
                            outs=[],
                            sync_info=mybir.SyncInfo(on_wait=[w],
                                                     on_update=[])))
                        n += 1
                    si.on_wait = [waits[-1]]
                    changed = True
                out.append(inst)
            if changed:
                blk.instructions = out
    return n


def _build(legalize=True):
    key = "nc" if legalize else "nc_raw"
    if key in _CACHE:
        return _CACHE[key]
    nc = bass.Bass("TRN2", target_bir_lowering=False, debug=False)

    # hi/lo fp8 planes packed into single DRAM tensors: every DMA dispatch
    # serializes through one shared HWDGE (~625ns), so fewer+wider wins
    h_d = nc.dram_tensor("h", [P, NTT, NPR, 2, 2, 512], FP8,
                         kind="ExternalInput").ap()
    w_d = nc.dram_tensor("w", [P, NF, NPR, 2, 2, P], FP8,
                         kind="ExternalInput").ap()
    wo_d = nc.dram_tensor("wo", [P, 2, 2, 2, D], FP8,
                          kind="ExternalInput").ap()
    cw_d = nc.dram_tensor("conv_w", [P, NF * 4], F32,
                          kind="ExternalInput").ap()
    out_d = nc.dram_tensor("out", [S, D], BF16, kind="ExternalOutput").ap()

    out_v = out_d.rearrange("(po pi) d -> pi po d", pi=P)      # [128,16,2048]

    with tile.TileContext(nc) as tc, ExitStack() as ctx:
        const = ctx.enter_context(tc.tile_pool(name="const", bufs=1))
        p_ht = ctx.enter_context(tc.tile_pool(name="ht", bufs=2))
        p_work = ctx.enter_context(tc.tile_pool(name="work", bufs=3))
        p_probs = ctx.enter_context(tc.tile_pool(name="probs", bufs=6))
        p_out = ctx.enter_context(tc.tile_pool(name="outp", bufs=4))
        ps2 = ctx.enter_context(tc.tile_pool(name="ps2", bufs=2, space="PSUM"))
        ps_s = ctx.enter_context(tc.tile_pool(name="ps_s", bufs=3,
                                              space="PSUM"))
        ps3 = ctx.enter_context(tc.tile_pool(name="ps3", bufs=2, space="PSUM"))
        ps1 = ctx.enter_context(tc.tile_pool(name="ps1", bufs=1, space="PSUM"))

        # --- constants / persistent tensors ---
        # tiny warm operand memset FIRST on Pool (~100ns) so the PE p-state
        # warm-up starts ~1us before make_identity would allow
        wtiny = const.tile([P, P], BF16, tag="wtiny")
        nc.gpsimd.memset(wtiny, 0.5)
        ident = const.tile([P, P], BF16, tag="ident")
        make_identity(nc, ident)
        wscr = const.tile([P, 512], BF16, tag="wscr")
        nc.vector.memset(wscr, 0.5)
        # PE p-state warm-up: the tensor engine runs at half clock until it
        # has been continuously busy 3us; burn that ramp on dummies while
        # the first input DMAs are still in flight.
        wps = ps_s.tile([P, 512], F32, tag="s", name="warm")
        for _ in range(8):
            nc.tensor.matmul(wps[:, 0:P], lhsT=wtiny, rhs=wtiny,
                             start=True, stop=True)
        for _ in range(WARM):
            nc.tensor.matmul(wps, lhsT=wtiny, rhs=wscr, start=True, stop=True)
        cw0 = const.tile([P, NF * 4], F32, tag="cw0")
        cw = const.tile([P, NF * 4], F32, tag="cw")
        wq = const.tile([P, NF, NPR, 2, 2, P], FP8, tag="wq")
        wo = const.tile([P, 2, 2, 2, D], FP8, tag="wo")
        # raw (pre-conv) qkv.T in bf16 (64x scale), 3 leading zero columns so
        # the causal conv taps can read t-3..t-1 without edge cases
        qkvf = const.tile([P, NF, S + 3], BF16, tag="qkvf")
        nc.gpsimd.memset(qkvf[:, :, 0:3], 0.0)
        qkvb = const.tile([P, NF, S], BF16, tag="qkvb")    # conv'd qkv.T
        vnat = const.tile([P, 16, P], BF16, tag="vnat")    # v in [token, dh]
        atth = const.tile([P, 4, S], FP8, tag="atth")      # attT hi per head
        attl = const.tile([P, 4, S], FP8, tag="attl")      # attT lo per head
        ones2 = const.tile([P, P], F16, tag="ones2")
        nc.vector.memset(ones2, 4.0)
        ebias = const.tile([P, 1], F32, tag="ebias")
        nc.vector.memset(ebias, EXP_BIAS)

        obuf = {}  # t4 -> wide bf16 output tile (one DMA per token chunk)

        def oproj_trio(tt16, op, dt, pr_, k0):
            # atth terms first: the attl piece lands one DVE op later in
            # the finalize chain, so leading with atth starts ~190ns sooner
            for k, (lhs, hl) in enumerate(
                    ((atth, 0), (atth, 1), (attl, 0))):
                nc.tensor.matmul(
                    op,
                    lhsT=lhs[:, ds(2 * pr_, 2), ds(tt16 * P, P)],
                    rhs=wo[:, hl, pr_, :, ds(dt * 512, 512)],
                    start=(k0 + k == 0), stop=(k0 + k == 5),
                    perf_mode=DR)

        def o_proj_final_pr0(t4):
            # head-0/1 trios of a final chunk: eligible as soon as the
            # first head pair is normalized, used as PE filler while the
            # last head's denominator drains on DVE
            tt16 = (NQT - 1) * 4 + t4
            ops = {}
            for dt in range(4):
                if dt % 2 == 1:
                    ops[dt] = ps_s.tile([P, 512], F32, tag="s", name="op")
                else:
                    ops[dt] = ps2.tile([P, 512], F32, tag="proj", name="op")
                oproj_trio(tt16, ops[dt], dt, 0, 0)
            return ops

        def o_proj_chunk(qt, t4, final=False, dts=(0, 1, 2, 3),
                         pre_ops=None):
            # output projection for one token-128-tile of q-tile qt; the
            # PSUM->SBUF copies land in one wide bf16 tile which ships as a
            # SINGLE DMA per token chunk (HWDGE dispatch is the scarce
            # resource, not DMA bandwidth)
            tt16 = qt * 4 + t4
            if t4 not in obuf:
                obuf[t4] = p_out.tile([P, 4, 512], BF16, tag="ob", name="ob")
            ob = obuf[t4]

            def trio(op, dt, pr_, k0):
                oproj_trio(tt16, op, dt, pr_, k0)

            for dt in dts:
                if pre_ops is not None:
                    op = pre_ops[dt]
                else:
                    if final and dt % 2 == 1:
                        op = ps_s.tile([P, 512], F32, tag="s", name="op")
                    else:
                        op = ps2.tile([P, 512], F32, tag="proj", name="op")
                    trio(op, dt, 0, 0)
                trio(op, dt, 1, 3)
                if final:
                    # alternate engines so the last chunk's copies overlap;
                    # ship in pieces so earlier DMAs overlap later matmuls
                    if dt == 3 and t4 == 3:
                        # the very last copy splits across ACT+DVE halves
                        # running in parallel (~390ns instead of 612)
                        nc.scalar.copy(ob[:, dt, ds(0, 256)],
                                       op[:, ds(0, 256)])
                        nc.vector.tensor_copy(ob[:, dt, ds(256, 256)],
                                              op[:, ds(256, 256)])
                    elif dt % 2 == 0:
                        nc.vector.tensor_copy(ob[:, dt, :], op)
                    else:
                        nc.scalar.copy(ob[:, dt, :], op)
                    if dt == 1:
                        nc.sync.dma_start(out_v[:, tt16, ds(0, 1024)],
                                          ob[:, ds(0, 2)])
                    elif dt == 2:
                        nc.sync.dma_start(out_v[:, tt16, ds(1024, 512)],
                                          ob[:, ds(2, 1)])
                    elif dt == 3:
                        nc.sync.dma_start(out_v[:, tt16, ds(1536, 512)],
                                          ob[:, ds(3, 1)])
                elif dt == 3:
                    nc.scalar.copy(ob[:, dt, :], op)
                else:
                    nc.vector.tensor_copy(ob[:, dt, :], op)
            if dts[-1] == 3:
                if not final:
                    nc.sync.dma_start(out_v[:, tt16, :], ob)
                del obuf[t4]

        def attn_B(qt, fillers=None, drain_filler=None):
            # attention for q-tile qt (needs token tiles <= qt). The four
            # heads are software-pipelined into ONE flat (h, kt) sequence:
            # scores of head h+1 are emitted while head h's attended matmuls
            # drain, so the in-order PE queue never stalls on the
            # exp (ACT) -> mask (Pool) producer chain. The previous q-tile's
            # output projection is interleaved as additional PE filler.
            nk = 4 * (qt + 1)
            LAG = 14
            state = {}  # h -> (att, colsum)
            fin = {}    # h -> (att, colsum) awaiting denominator finalize
            fin_q = deque()  # [h, consumes-since-ready]
            pend = deque()
            pr_quad = None

            def consume():
                ch, ppr, px0, pkt = pend.popleft()
                att, colsum = state[ch] if ch in state else fin[ch]
                nc.tensor.matmul(
                    att[:, px0:512], lhsT=vnat[:, pkt, :],
                    rhs=ppr[:, px0:512],
                    start=(pkt == 0), stop=(pkt == nk - 1))
                # softmax denominator: accumulate exp'd probs on DVE
                # (partition dim reduced by ONE ones-matmul at the end)
                if pkt == 0:
                    nc.vector.tensor_copy(colsum, ppr)
                else:
                    nc.vector.tensor_add(
                        colsum[:, px0:512], colsum[:, px0:512],
                        ppr[:, px0:512])
                if pkt == nk - 1:
                    fin[ch] = state.pop(ch)
                    fin_q.append([ch, 0])

            def finalize(ch):
                att, colsum = fin[ch]
                smp = ps1.tile([P, 512], F32, tag="small")
                rec = p_work.tile([P, 512], F32, tag="rec")
                t16 = p_work.tile([P, 512], F16, tag="t16")
                if qt == NQT - 1 and ch >= 2:
                    # last heads before the final output projection: the
                    # whole normalization chain runs in 128-col pieces
                    # (ones-matmul and reciprocal included) so the first
                    # final o_proj trios start ~0.6us sooner
                    for pc in range(4):
                        c = ds(pc * P, P)
                        nc.tensor.matmul(smp[:, c], lhsT=ones2,
                                         rhs=colsum[:, c],
                                         start=True, stop=True)
                        nc.vector.reciprocal(rec[:, c], smp[:, c])
                        nc.vector.tensor_mul(t16[:, c], att[:, c], rec[:, c])
                        nc.vector.tensor_copy(
                            atth[:, ch, ds(qt * 512 + pc * P, P)], t16[:, c])
                        nc.vector.tensor_sub(
                            attl[:, ch, ds(qt * 512 + pc * P, P)], t16[:, c],
                            atth[:, ch, ds(qt * 512 + pc * P, P)])
                else:
                    nc.tensor.matmul(smp, lhsT=ones2, rhs=colsum,
                                     start=True, stop=True)
                    nc.vector.reciprocal(rec, smp)
                    nc.vector.tensor_mul(t16, att, rec)
                    nc.gpsimd.tensor_copy(atth[:, ch, ts(qt, 512)], t16)
                    nc.gpsimd.tensor_sub(attl[:, ch, ts(qt, 512)], t16,
                                         atth[:, ch, ts(qt, 512)])
                del fin[ch]

            for h in range(4):
                if fillers and h in fillers:
                    fillers[h]()
                state[h] = (ps3.tile([P, 512], F32, tag="att", name="att"),
                            p_work.tile([P, 512], F16, tag="colsum",
                                        name="colsum"))
                for kt in range(nk):
                    # shorter pending queue only near the END of the very
                    # last head: early kts keep the deep pipeline (PE ahead
                    # of exp), the tail still drains early so the final
                    # output projection starts with less latency
                    lag = 4 if (qt == NQT - 1 and h == 3
                                and kt >= nk - 6) else LAG
                    # previous q-tile's output projection emitted mid-head;
                    # on long tiles spread it at four points so the filler
                    # matches the exp-bound score/attend cadence
                    if qt > 0:
                        pts = {8: (2, 4, 5, 7), 12: (3, 6, 9, 11),
                               16: (4, 8, 12, 15)}[nk]
                        if kt in pts:
                            o_proj_chunk(qt - 1, h, dts=(pts.index(kt),))
                    j = kt - 4 * qt
                    x0 = j * P if j >= 0 else 0
                    F = 512 - x0
                    sp = ps_s.tile([P, 512], F32, tag="s")
                    nc.tensor.matmul(
                        sp[:, x0:512],
                        lhsT=qkvb[:, 4, ds(kt * P, P)],
                        rhs=qkvb[:, h, ds(qt * 512 + x0, F)],
                        start=True, stop=True,
                    )
                    if kt % 4 == 0:
                        pr_quad = p_probs.tile([P, 4, 512], BF16, tag="probs")
                    pr = pr_quad[:, kt % 4, :]
                    nc.scalar.activation(
                        pr[:, x0:512], sp[:, x0:512],
                        mybir.ActivationFunctionType.Exp,
                        scale=EXP_SCALE, bias=ebias)
                    if j >= 0:
                        # zero the k>q half of the diagonal tile in place
                        # (local col c vs partition p: keep iff c >= p)
                        nc.gpsimd.affine_select(
                            out=pr[:, x0:512], in_=pr[:, x0:512],
                            pattern=[[1, F]], base=0,
                            channel_multiplier=-1,
                            compare_op=mybir.AluOpType.is_ge, fill=0.0)
                    pend.append((h, pr, x0, kt))
                    thr = 1 if qt == NQT - 1 else 3
                    while len(pend) > lag:
                        consume()
                        for e in fin_q:
                            e[1] += 1
                        if fin_q and fin_q[0][1] >= thr:
                            finalize(fin_q.popleft()[0])
            while pend:
                consume()
            if drain_filler is not None:
                # PE work emitted ahead of the remaining finalizes, whose
                # ones-matmuls stall in-order on the DVE colsum chain
                drain_filler()
            while fin_q:
                finalize(fin_q.popleft()[0])

        # ------- Fused phases: per token tile: projection+conv, then the
        # attention q-tile that just became computable, then the (pipelined)
        # output projection of the previous q-tile.
        def ht_alloc():
            return p_ht.tile([P, NPR, 2, 2, 512], FP8, tag="ht", name="ht")

        def ht_dispatch(ht, tt, chunks):
            for c0, w_ in chunks:
                nc.sync.dma_start(ht[:, ds(c0, w_)],
                                  h_d[:, tt, ds(c0, w_)])

        def conv_fc(tt, fc, pp, dve_copy=False):
            t0 = tt * 512
            # pre-conv x (64x) -> bf16 for the DVE conv taps. When run
            # as attention filler, the copy goes on DVE because ACT is
            # clogged with exp tiles there.
            if dve_copy:
                nc.vector.tensor_copy(qkvf[:, fc, ds(3 + t0, 512)], pp)
            else:
                nc.scalar.copy(qkvf[:, fc, ds(3 + t0, 512)], pp)
            # conv taps: out[t] = x[t] + sum_k x[t+k-3]*w[k].
            # Products via tensor_scalar (4x DVE mode — the tensor-tensor
            # variant gets no fast mode), sums via bf16 tensor_tensor (2x)
            ca = p_work.tile([P, 512], BF16, tag="ctmpa", name="ca")
            cb = p_work.tile([P, 512], BF16, tag="ctmpb", name="cb")
            nc.vector.tensor_scalar(
                ca, qkvf[:, fc, ds(t0 + 0, 512)],
                cw[:, fc * 4 + 0: fc * 4 + 1], None, op0=MULT)
            nc.vector.tensor_scalar(
                cb, qkvf[:, fc, ds(t0 + 1, 512)],
                cw[:, fc * 4 + 1: fc * 4 + 2], None, op0=MULT)
            nc.vector.tensor_add(ca, ca, cb)
            nc.vector.tensor_scalar(
                cb, qkvf[:, fc, ds(t0 + 2, 512)],
                cw[:, fc * 4 + 2: fc * 4 + 3], None, op0=MULT)
            nc.vector.tensor_add(ca, ca, cb)
            # last tap's weight is pre-biased +1 on the host, folding the
            # residual x[t] into the same tensor_scalar product
            nc.vector.tensor_scalar(
                cb, qkvf[:, fc, ds(t0 + 3, 512)],
                cw[:, fc * 4 + 3: fc * 4 + 4], None, op0=MULT)
            nc.vector.tensor_add(qkvb[:, fc, ts(tt, 512)], ca, cb)

        def proj_fc(tt, fc, ht, dve_copy=False):
            pp = ps2.tile([P, 512], F32, tag="proj", name="pp")
            # pr-major: each hidden pr chunk is fully consumed (all three
            # hi/lo terms) as soon as it lands, minimizing startup stalls
            k = 0
            for pr_ in range(NPR):
                for whl, hhl in ((0, 0), (0, 1), (1, 0)):
                    nc.tensor.matmul(
                        pp, lhsT=wq[:, fc, pr_, whl], rhs=ht[:, pr_, hhl],
                        start=(k == 0), stop=(k == 3 * NPR - 1),
                        perf_mode=DR)
                    k += 1
            conv_fc(tt, fc, pp, dve_copy)

        def transp_v(tt):
            # v (fc=5) of this token tile -> natural [token, dh] layout
            trp = ps1.tile([P, 512], BF16, tag="small")
            for j in range(4):
                nc.tensor.transpose(trp[:, ds(j * P, P)],
                                    qkvb[:, 5, ds((tt * 4 + j) * P, P)],
                                    ident)
            nc.vector.tensor_copy(vnat[:, ds(tt * 4, 4), :], trp)

        prefetched = None
        for tt in range(NTT):
            if tt == 0:
                # The DMA engine pool executes one transfer at a time, so
                # the dispatch sequence below is a global priority schedule:
                # interleave hidden pr-chunks with the weight packs in
                # exactly the order the paired pr-major projection consumes
                # them (fc4+fc5 first, pr chunk by pr chunk).
                ht = ht_alloc()
                ht_dispatch(ht, 0, ((0, 1),))
                nc.sync.dma_start(wq[:, 4, ds(0, 4)], w_d[:, 4, ds(0, 4)])
                nc.sync.dma_start(wq[:, 5, ds(0, 4)], w_d[:, 5, ds(0, 4)])
                ht_dispatch(ht, 0, ((1, 1), (2, 2)))
                nc.sync.dma_start(wq[:, 4, ds(4, 4)], w_d[:, 4, ds(4, 4)])
                nc.sync.dma_start(wq[:, 5, ds(4, 4)], w_d[:, 5, ds(4, 4)])
                ht_dispatch(ht, 0, ((4, 2), (6, 2)))
                nc.sync.dma_start(cw0, cw_d)
                nc.vector.tensor_copy(cw, cw0)
                for fc in (0, 1, 2, 3):
                    nc.sync.dma_start(wq[:, fc], w_d[:, fc])
                prefetched = ht_alloc()
                ht_dispatch(prefetched, 1, ((0, 4), (4, 4)))
            else:
                ht = prefetched
                if tt == 1:
                    nc.sync.dma_start(wo, wo_d)
                if tt < NTT - 1:
                    prefetched = ht_alloc()
                    ht_dispatch(prefetched, tt + 1, ((0, 4), (4, 4)))

            if tt == 0:
                # fc4+fc5 chains interleaved pr-major: two PSUM chains
                # consume each hidden pr chunk at the DMA supply rate; the
                # remaining projections ride inside the attention blocks
                pps = {4: ps2.tile([P, 512], F32, tag="proj", name="pp"),
                       5: ps2.tile([P, 512], F32, tag="proj", name="pp")}
                ks = {4: 0, 5: 0}
                for pr_ in range(NPR):
                    for fc in (4, 5):
                        for whl, hhl in ((0, 0), (0, 1), (1, 0)):
                            nc.tensor.matmul(
                                pps[fc], lhsT=wq[:, fc, pr_, whl],
                                rhs=ht[:, pr_, hhl],
                                start=(ks[fc] == 0),
                                stop=(ks[fc] == 3 * NPR - 1), perf_mode=DR)
                            ks[fc] += 1
                    # pad known DMA-supply stalls with warm matmuls so the
                    # PE p-state ramp survives the bandwidth-gated stretch
                    for _ in range({1: 2, 3: 4}.get(pr_, 0)):
                        wpad = ps_s.tile([P, 512], F32, tag="s", name="wpad")
                        nc.tensor.matmul(wpad, lhsT=wtiny, rhs=wscr,
                                         start=True, stop=True)
                conv_fc(0, 4, pps[4])
                conv_fc(0, 5, pps[5])
                proj_fc(0, 0, ht)
                proj_fc(0, 1, ht)
            else:
                for fc in (0, 1, 2, 3):
                    proj_fc(tt, fc, ht)
            transp_v(tt)
            # the next tile's k/v projections (fc4, fc5) ride as PE filler
            # inside this tile's attention: they only need the prefetched
            # hidden tile, and attention's exp-bound stretches absorb them
            fillers = {}
            if tt == 0:
                fillers[0] = lambda: proj_fc(0, 2, ht, dve_copy=True)
                fillers[1] = lambda: proj_fc(0, 3, ht, dve_copy=True)
            if tt < NTT - 1:
                nxt, pf = tt + 1, prefetched
                fillers[1 if tt else 2] = \
                    lambda n=nxt, p=pf: proj_fc(n, 4, p, dve_copy=True)
                fillers[3] = lambda n=nxt, p=pf: proj_fc(n, 5, p,
                                                         dve_copy=True)
            final_pr0 = {}
            if tt == NTT - 1:
                def drain_filler():
                    final_pr0[0] = o_proj_final_pr0(0)
                attn_B(tt, fillers=fillers, drain_filler=drain_filler)
            else:
                attn_B(tt, fillers=fillers)
        for t4 in range(4):
            o_proj_chunk(NQT - 1, t4, final=True,
                         pre_ops=final_pr0.get(t4))

    if legalize:
        _legalize_waits(nc)
    _CACHE[key] = nc
    return nc


def _prep_inputs(hidden_states, w_q, w_k, w_v, w_o, conv_w):
    """Build the 8 per-core input maps (host-side shard + fp8 hi/lo split)."""
    f8 = ml_dtypes.float8_e4m3

    def hpairs(x):  # [2048 d, 2048 t] -> [128, 4, 8, 2, 512]
        return np.ascontiguousarray(
            x.reshape(NPR, 2, P, NTT, 512).transpose(2, 3, 0, 1, 4))

    def wpairs(x):  # [2048, 768] -> [128, 6, 8, 2, 128]
        return np.ascontiguousarray(
            x.reshape(NPR, 2, P, NF, P).transpose(2, 3, 0, 1, 4))

    def split8(x):
        hi = x.astype(f8)
        lo = (x - hi.astype(np.float32)).astype(f8)
        return hi, lo

    # hidden split is shared by the 4 cores of a batch
    h_packs = []
    for b in range(2):
        hT = np.ascontiguousarray(hidden_states[b].T)
        hi, lo = split8(hT)
        h_packs.append(np.ascontiguousarray(
            np.stack([hpairs(hi), hpairs(lo)], axis=3)))

    in_maps = []
    for c in range(8):
        b, g = c // 4, c % 4
        wqkv = np.concatenate(
            [w_q[:, g * 512:(g + 1) * 512],
             w_k[:, g * 128:(g + 1) * 128],
             w_v[:, g * 128:(g + 1) * 128]], axis=1) * WSCALE
        w_hi, w_lo = split8(wqkv)
        w_pack = np.ascontiguousarray(
            np.stack([wpairs(w_hi), wpairs(w_lo)], axis=3))
        wop = np.ascontiguousarray(w_o[g * 512:(g + 1) * 512, :]) * WSCALE
        wo_hi, wo_lo = split8(wop)
        wo_pack = np.ascontiguousarray(np.stack(
            [wo_hi.reshape(2, 2, P, D).transpose(2, 0, 1, 3),
             wo_lo.reshape(2, 2, P, D).transpose(2, 0, 1, 3)], axis=1))
        cwc = np.concatenate(
            [conv_w[g * 512:(g + 1) * 512],
             conv_w[2048 + g * 128: 2048 + (g + 1) * 128],
             conv_w[2560 + g * 128: 2560 + (g + 1) * 128]], axis=0)  # [768,4]
        cwp = np.ascontiguousarray(
            cwc.reshape(NF, P, 4).transpose(1, 0, 2).reshape(P, NF * 4)
        ).astype(np.float32)
        # residual fold: out = x + sum_k x_k w_k == sum taps with w3 += 1
        cwp[:, 3::4] += 1.0
        in_maps.append({
            "h": h_packs[b],
            "w": w_pack,
            "wo": wo_pack,
            "conv_w": cwp,
        })
    return in_maps


def kernel(hidden_states, w_q, w_k, w_v, w_o, conv_w, _trace=False):
    nc = _build()
    in_maps = _prep_inputs(
        np.asarray(hidden_states, dtype=np.float32),
        np.asarray(w_q, dtype=np.float32),
        np.asarray(w_k, dtype=np.float32),
        np.asarray(w_v, dtype=np.float32),
        np.asarray(w_o, dtype=np.float32),
        np.asarray(conv_w, dtype=np.float32),
    )
    res = run_bass_kernel_spmd(nc, in_maps, core_ids=list(range(8)),
                               trace=_trace)
    outs = [r["out"] for r in res.results]
    full = np.empty((2, S, D), dtype=np.float32)
    for b in range(2):
        acc = (outs[4 * b].astype(np.float32)
               + outs[4 * b + 1].astype(np.float32)
               + outs[4 * b + 2].astype(np.float32)
               + outs[4 * b + 3].astype(np.float32))
        full[b] = acc * (1.0 / OUT_DIV)
    if _trace:
        kernel.last_results = res
    return full
